# revision 18
# baseline (speedup 1.0000x reference)
"""Trainium2 Bass kernel for nn_Attention_84215718740239 (sparse attention
with Gumbel top-k mask dropout).

Strategy: data-parallel over batch (2 batches/core x 8 cores = 16 (b,h)
slices per core). All compute on-device except:
  - host-side layout prep (transposes of x / weights for DMA-friendly APs)
  - the Gumbel noise table, which is a data-independent constant
    (jax.random.key(42)) precomputed once on host
  - final un-shard (transpose back + bias add)

Device pipeline per core:
  B: qkv projection -> qT,kT (d-major, f32) and v (natural, bf16)
  C: per slice: attn logits (PE, f32) -> exp/stats/log-prob -> scores
     (scores = log(spatial)+log(aqm)+gumbel, diag pre-killed via the
     gumbel constant)
  D: per-slice top-k THRESHOLD via 6 count-sweeps (2 fixed probes +
     4 secant updates); counts by tensor_scalar(is_ge, accum_out) +
     PE ones-matmul partition reduction
  E: masked softmax applied multiplicatively (E2 = E * (score < thr)),
     DMA-transpose E2 (bf16), attn@v on PE, rows scaled by 1/rowsum
  F: output projection (bf16) producing out^T; host transposes back
"""

import numpy as np

import sys
if '/opt/trn_rl_repo' not in sys.path:
    sys.path.insert(0, '/opt/trn_rl_repo')

import concourse.bass as bass
import concourse.tile as tile
from concourse import bacc, mybir
from concourse.bass_utils import run_bass_kernel_spmd

F32 = mybir.dt.float32
BF16 = mybir.dt.bfloat16
AX = mybir.AxisListType
OP = mybir.AluOpType
AF = mybir.ActivationFunctionType

B, N, C, H, HD = 16, 512, 512, 8, 64
NCORES = 8
BPC = B // NCORES            # batches per core
NSL = BPC * H                # 16 slices per core
LF = N // 2                  # 256
KTARG = float(int(0.1 * (LF - 1)) * N)   # 12800 samples per slice
TOK = BPC * N                # 1024 tokens per core

_gum_cache = None


def _gumbel_full():
    """[128, 512, 256] f32 Gumbel noise (fixed key 42), diag pre-set to -1e30."""
    global _gum_cache
    if _gum_cache is None:
        import jax
        import jax.numpy as jnp
        with jax.default_device(jax.devices('cpu')[0]):
            u = jax.random.uniform(jax.random.key(42), (B * H, N * LF),
                                   dtype=jnp.float32, minval=1e-20, maxval=1.0)
            g = np.asarray(-jnp.log(-jnp.log(u))).reshape(B * H, N, LF).copy()
        idx = np.arange(LF)
        g[:, idx, idx] = -1e30
        g[:, LF + idx, idx] = -1e30
        _gum_cache = g
    return _gum_cache


def _eye_mask():
    e = np.zeros((2, 128, LF), np.float32)
    p = np.arange(128)
    e[0, p, p] = 1.0
    e[1, p, 128 + p] = 1.0
    return e


_nc_cache = None
DEBUG_DUMP = False


def _build():
    global _nc_cache
    if _nc_cache is not None:
        return _nc_cache
    nc = bacc.Bacc("TRN2", target_bir_lowering=False, debug=False,
                   num_devices=NCORES)

    xt_d = nc.dram_tensor("xt", [C, TOK], F32, kind="ExternalInput").ap()
    wq_d = nc.dram_tensor("wqt", [C, 3 * C], F32, kind="ExternalInput").ap()
    wp_d = nc.dram_tensor("wpt", [C, C], F32, kind="ExternalInput").ap()
    gum_d = nc.dram_tensor("gum", [NSL, N, LF], F32, kind="ExternalInput").ap()
    eye_d = nc.dram_tensor("eye", [2, 128, LF], F32, kind="ExternalInput").ap()
    out_d = nc.dram_tensor("outt", [C, TOK], F32, kind="ExternalOutput").ap()

    dbg = None
    if DEBUG_DUMP:
        dbg = {}
        for name, shape in DEBUG_DUMP.items():
            dbg[name] = nc.dram_tensor(name, shape, F32,
                                       kind="ExternalOutput").ap()

    with tile.TileContext(nc) as tc:
        _emit(nc, tc, xt_d, wq_d, wp_d, gum_d, eye_d, out_d, dbg)
    nc.compile()
    _nc_cache = nc
    return nc


def _emit(nc, tc, xt_d, wq_d, wp_d, gum_d, eye_d, out_d, dbg=None):
    from contextlib import ExitStack
    ctx = ExitStack()
    with ctx:
        const = ctx.enter_context(tc.tile_pool(name="const", bufs=1))
        stats = ctx.enter_context(tc.tile_pool(name="stats", bufs=1))

        # ---------- load inputs ----------
        eye = const.tile([128, 2, LF], F32)
        nc.sync.dma_start(eye[:], eye_d.rearrange("e p c -> p e c"))

        ones_col = const.tile([128, 1], F32)      # lhsT for partition-sum
        nc.vector.memset(ones_col[:], 1.0)
        ones_row = const.tile([1, 128], F32)      # lhsT for partition-bcast
        nc.vector.memset(ones_row[:], 1.0)
        one16 = const.tile([1, NSL], F32)
        nc.vector.memset(one16[:], 1.0)
        c1em6 = const.tile([128, 1], F32)
        nc.vector.memset(c1em6[:], 1e-6)

        wpb = const.tile([128, 4, C], BF16)
        vbf = const.tile([128, 8, C], BF16)       # v natural [tok, vc]
        so_all = stats.tile([128, NSL, 4], F32)   # opp-half exp row-sums
        thr = stats.tile([128, NSL], F32)         # bcast thresholds

        qkpool = ctx.enter_context(tc.tile_pool(name="qkp", bufs=1))
        qk = qkpool.tile([128, 8, TOK], F32)      # q^T,k^T [outc, tok]

        # ---------- phase B: qkv projection ----------
        with tc.tile_pool(name="pbps", bufs=4, space="PSUM") as pb, \
             tc.tile_pool(name="pin", bufs=1) as pin:
            xt = pin.tile([128, 4, TOK], F32)     # x^T  [c, tok]
            nc.sync.dma_start(xt[:], xt_d.rearrange("(k p) t -> p k t", p=128))
            wq = pin.tile([128, 4, 3 * C], F32)   # w_qkv^T [c, outc]
            nc.sync.dma_start(wq[:], wq_d.rearrange("(k p) t -> p k t", p=128))
            wp = pin.tile([128, 4, C], F32)       # w_proj^T [c, oc]
            nc.sync.dma_start(wp[:], wp_d.rearrange("(k p) t -> p k t", p=128))
            for k in range(4):
                nc.vector.tensor_copy(wpb[:, k, :], wp[:, k, :])
            for po in range(8):                   # outc tiles of q,k
                for tch in range(2):
                    ps = pb.tile([128, 512], F32)
                    for kc in range(4):
                        nc.tensor.matmul(ps[:], wq[:, kc, bass.ts(po, 128)],
                                         xt[:, kc, bass.ts(tch, 512)],
                                         start=(kc == 0), stop=(kc == 3))
                    if (po + tch) % 2:
                        nc.scalar.copy(qk[:, po, bass.ts(tch, 512)], ps[:])
                    else:
                        nc.vector.tensor_copy(qk[:, po, bass.ts(tch, 512)],
                                              ps[:])
            for tt in range(8):                   # v tok tiles
                ps = pb.tile([128, 512], F32)
                for kc in range(4):
                    nc.tensor.matmul(ps[:], xt[:, kc, bass.ts(tt, 128)],
                                     wq[:, kc, bass.ds(2 * C, 512)],
                                     start=(kc == 0), stop=(kc == 3))
                if tt % 2:
                    nc.scalar.copy(vbf[:, tt, :], ps[:])
                else:
                    nc.vector.tensor_copy(vbf[:, tt, :], ps[:])

        # persistent big tensors (allocated after phase-B scratch is freed)
        big = ctx.enter_context(tc.tile_pool(name="big", bufs=1))
        ebf = big.tile([128, NSL, 4, N], BF16)    # exp(attn) per slice/tile
        sc = big.tile([128, NSL, 4 * LF], F32)    # scores per slice

        # ---------- phase C: logits -> scores per slice ----------
        with tc.tile_pool(name="cps", bufs=4, space="PSUM") as cp, \
             tc.tile_pool(name="ef", bufs=5) as efp, \
             tc.tile_pool(name="gp", bufs=2) as gp, \
             tc.tile_pool(name="st4", bufs=4) as st4, \
             tc.tile_pool(name="gj", bufs=2) as gjp:
            for s in range(NSL):
                b, h = divmod(s, H)
                qpo, qpp = h // 2, 64 * (h % 2)
                gt = gp.tile([128, 4, LF], F32, tag="g")
                nc.sync.dma_start(
                    gt[:], gum_d[s].rearrange("(t p) c -> p t c", p=128))
                ssr4 = st4.tile([128, 4], F32, tag="ssr")
                so4v = so_all[:, s, :]
                eii4 = st4.tile([128, 4], F32, tag="eii")
                mx4 = st4.tile([128, 4], F32, tag="mx")
                efs = []
                for t in range(4):
                    soff = 0 if t < 2 else LF
                    ooff = LF - soff
                    ps = cp.tile([128, 512], F32, tag="attn")
                    lhs = qk[qpp:qpp + 64, qpo, bass.ds(b * N + t * 128, 128)]
                    rhs = qk[qpp:qpp + 64, 4 + qpo, bass.ds(b * N, 512)]
                    nc.tensor.matmul(ps[:], lhs, rhs, start=True, stop=True)
                    ef = efp.tile([128, 512], F32, tag="ef")
                    efs.append(ef)
                    # row max of opposite block (logit domain, f32)
                    nc.vector.tensor_reduce(mx4[:, t:t + 1],
                                            ps[:, ooff:ooff + LF],
                                            axis=AX.X, op=OP.max)
                    nc.scalar.activation(ef[:, soff:soff + LF],
                                         ps[:, soff:soff + LF], AF.Exp,
                                         accum_out=ssr4[:, t:t + 1])
                    nc.scalar.activation(ef[:, ooff:ooff + LF],
                                         ps[:, ooff:ooff + LF], AF.Exp,
                                         accum_out=so4v[:, t:t + 1])
                    # diagonal element of the same-block
                    gj = gjp.tile([128, LF], F32, tag="gj")
                    nc.vector.scalar_tensor_tensor(
                        gj[:], ef[:, soff:soff + LF], 1.0, eye[:, t % 2, :],
                        op0=OP.mult, op1=OP.mult,
                        accum_out=eii4[:, t:t + 1])
                    nc.vector.tensor_copy(ebf[:, s, t, :], ef[:])
                # per-row scalars for all 4 tiles at once  [128,4]
                d4 = st4.tile([128, 4], F32, tag="d4")
                nc.vector.tensor_tensor(d4[:], ssr4[:], so4v, op=OP.add)
                rd4 = st4.tile([128, 4], F32, tag="rd4")
                nc.vector.reciprocal(rd4[:], d4[:])
                sv4 = st4.tile([128, 4], F32, tag="sv4")
                nc.vector.tensor_tensor(sv4[:], ssr4[:], eii4[:],
                                        op=OP.subtract)
                nc.vector.tensor_tensor(sv4[:], sv4[:], rd4[:], op=OP.mult)
                nc.vector.tensor_scalar(sv4[:], sv4[:], float((LF - 1) * 1e-6),
                                        None, op0=OP.add)
                aqm4 = st4.tile([128, 4], F32, tag="aqm")
                nc.scalar.activation(aqm4[:], mx4[:], AF.Exp)
                nc.vector.tensor_tensor(aqm4[:], aqm4[:], rd4[:], op=OP.mult)
                nc.vector.tensor_scalar(aqm4[:], aqm4[:], 1e-6, None,
                                        op0=OP.add)
                c4 = st4.tile([128, 4], F32, tag="c4")
                nc.scalar.activation(c4[:], aqm4[:], AF.Ln)
                lns = st4.tile([128, 4], F32, tag="lns")
                nc.scalar.activation(lns[:], sv4[:], AF.Ln)
                nc.vector.tensor_tensor(c4[:], c4[:], lns[:], op=OP.subtract)
                for t in range(4):
                    soff = 0 if t < 2 else LF
                    slot = sc[:, s, bass.ts(t, LF)]
                    nc.scalar.activation(slot, efs[t][:, soff:soff + LF],
                                         AF.Ln, scale=rd4[:, t:t + 1],
                                         bias=c1em6[:])
                    nc.vector.scalar_tensor_tensor(
                        slot, slot, c4[:, t:t + 1], gt[:, t, :],
                        op0=OP.add, op1=OP.add)

        # ---------- phase D: threshold selection (secant, 6 sweeps) ----------
        with tc.tile_pool(name="dps", bufs=2, space="PSUM") as dps, \
             tc.tile_pool(name="dbc", bufs=2, space="PSUM") as dbc, \
             tc.tile_pool(name="sel", bufs=1) as sel, \
             tc.tile_pool(name="junk", bufs=2) as jp, \
             tc.tile_pool(name="s16", bufs=8) as s16:
            cnt128 = sel.tile([128, NSL], F32)
            t0 = sel.tile([1, NSL], F32)
            t1 = sel.tile([1, NSL], F32)
            n0 = sel.tile([1, NSL], F32)
            n1 = sel.tile([1, NSL], F32)
            nc.vector.memset(t0[:], -9.10)
            nc.vector.memset(t1[:], -8.80)

            def count_into(ndst):
                for s in range(NSL):
                    jk = jp.tile([128, 4 * LF], BF16, tag="junk")
                    nc.vector.tensor_scalar(jk[:], sc[:, s, :],
                                            thr[:, s:s + 1], 0.0,
                                            op0=OP.is_ge, op1=OP.add,
                                            accum_out=cnt128[:, s:s + 1])
                cp_ = dps.tile([1, NSL], F32, tag="cnt")
                nc.tensor.matmul(cp_[:], ones_col[:], cnt128[:],
                                 start=True, stop=True)
                nc.vector.tensor_copy(ndst, cp_[:])

            def bcast(tsrc):
                bp = dbc.tile([128, NSL], F32, tag="bc")
                nc.tensor.matmul(bp[:], ones_row[:], tsrc, start=True,
                                 stop=True)
                nc.vector.tensor_copy(thr[:], bp[:])

            bcast(t0[:])
            count_into(n0[:])
            bcast(t1[:])
            count_into(n1[:])
            for r in range(4):
                d = s16.tile([1, NSL], F32, tag="d")
                nc.vector.tensor_tensor(d[:], n1[:], n0[:], op=OP.subtract)
                dsq = s16.tile([1, NSL], F32, tag="dsq")
                nc.vector.tensor_tensor(dsq[:], d[:], d[:], op=OP.mult)
                msk = s16.tile([1, NSL], F32, tag="msk")
                nc.vector.tensor_scalar(msk[:], dsq[:], 1.0, None,
                                        op0=OP.is_lt)
                nc.vector.tensor_tensor(d[:], d[:], msk[:], op=OP.subtract)
                rcd = s16.tile([1, NSL], F32, tag="rcd")
                nc.vector.reciprocal(rcd[:], d[:])
                dt = s16.tile([1, NSL], F32, tag="dt")
                nc.vector.tensor_tensor(dt[:], t1[:], t0[:], op=OP.subtract)
                nk = s16.tile([1, NSL], F32, tag="nk")
                nc.vector.tensor_scalar(nk[:], n1[:], KTARG, None,
                                        op0=OP.subtract)
                nc.vector.tensor_tensor(nk[:], nk[:], dt[:], op=OP.mult)
                nc.vector.tensor_tensor(nk[:], nk[:], rcd[:], op=OP.mult)
                t2 = s16.tile([1, NSL], F32, tag="t2")
                nc.vector.tensor_tensor(t2[:], t1[:], nk[:], op=OP.subtract)
                nc.vector.tensor_scalar(t2[:], t2[:], -8.0, -10.0,
                                        op0=OP.min, op1=OP.max)
                nc.vector.tensor_copy(t0[:], t1[:])
                nc.vector.tensor_copy(n0[:], n1[:])
                nc.vector.tensor_copy(t1[:], t2[:])
                bcast(t1[:])
                count_into(n1[:])
            if dbg is not None and "dbg_n" in dbg:
                nn = sel.tile([1, 2 * NSL], F32)
                nc.vector.tensor_copy(nn[:, :NSL], n0[:])
                nc.vector.tensor_copy(nn[:, NSL:], n1[:])
                nc.sync.dma_start(dbg["dbg_n"], nn[:])

        if dbg is not None:
            srcs = {"dbg_sc": sc, "dbg_thr": thr, "dbg_so": so_all,
                    "dbg_qk": qk}
            for name, t_ in srcs.items():
                if name in dbg:
                    nc.sync.dma_start(dbg[name], t_[:])

        # ---------- phase E: masked softmax + attn@v ----------
        obf = big.tile([128, 8, C], BF16)         # attn-out natural [tok, c]
        with tc.tile_pool(name="eps", bufs=4, space="PSUM") as ep, \
             tc.tile_pool(name="e2p", bufs=2) as e2p, \
             tc.tile_pool(name="e2tp", bufs=2) as e2tp, \
             tc.tile_pool(name="est", bufs=2) as est:
            for s in range(NSL):
                b, h = divmod(s, H)
                e2 = e2p.tile([128, 4, N], BF16, tag="e2")
                ss2 = est.tile([128, 4], F32, tag="ss2")
                for t in range(4):
                    soff = 0 if t < 2 else LF
                    ooff = LF - soff
                    nc.vector.scalar_tensor_tensor(
                        e2[:, t, soff:soff + LF], sc[:, s, bass.ts(t, LF)],
                        thr[:, s:s + 1], ebf[:, s, t, soff:soff + LF],
                        op0=OP.is_lt, op1=OP.mult,
                        accum_out=ss2[:, t:t + 1])
                    nc.vector.tensor_copy(e2[:, t, ooff:ooff + LF],
                                          ebf[:, s, t, ooff:ooff + LF])
                rd2 = est.tile([128, 4], F32, tag="rd2")
                nc.vector.tensor_tensor(rd2[:], ss2[:], so_all[:, s, :],
                                        op=OP.add)
                nc.vector.reciprocal(rd2[:], rd2[:])
                e2t = e2tp.tile([128, 4, N], BF16, tag="e2t")
                for u in range(4):
                    for t in range(4):
                        nc.sync.dma_start_transpose(
                            e2t[:, u, bass.ts(t, 128)],
                            e2[:, t, bass.ts(u, 128)])
                for t in range(4):
                    ps = ep.tile([128, 64], F32, tag="av")
                    for u in range(4):
                        nc.tensor.matmul(ps[:],
                                         e2t[:, u, bass.ts(t, 128)],
                                         vbf[:, 4 * b + u, bass.ts(h, 64)],
                                         start=(u == 0), stop=(u == 3))
                    nc.vector.tensor_scalar(obf[:, 4 * b + t, bass.ts(h, 64)],
                                            ps[:], rd2[:, t:t + 1], None,
                                            op0=OP.mult)

        # ---------- phase F: output projection ----------
        with tc.tile_pool(name="fps", bufs=4, space="PSUM") as fp, \
             tc.tile_pool(name="obt", bufs=1) as obtp, \
             tc.tile_pool(name="o2", bufs=1) as o2p:
            obt = obtp.tile([128, 4, TOK], BF16)  # out^T [c, tok]
            for ct in range(4):
                for tt in range(8):
                    nc.sync.dma_start_transpose(obt[:, ct, bass.ts(tt, 128)],
                                                obf[:, tt, bass.ts(ct, 128)])
            o2 = o2p.tile([128, 4, TOK], F32)     # proj out^T [oc, tok]
            for oc in range(4):
                for tch in range(2):
                    ps = fp.tile([128, 512], F32, tag="pj")
                    for ct in range(4):
                        nc.tensor.matmul(ps[:], wpb[:, ct, bass.ts(oc, 128)],
                                         obt[:, ct, bass.ts(tch, 512)],
                                         start=(ct == 0), stop=(ct == 3))
                    if (oc + tch) % 2:
                        nc.scalar.copy(o2[:, oc, bass.ts(tch, 512)], ps[:])
                    else:
                        nc.vector.tensor_copy(o2[:, oc, bass.ts(tch, 512)],
                                              ps[:])
            nc.sync.dma_start(out_d.rearrange("(k p) t -> p k t", p=128),
                              o2[:])


def _prep_inputs(x, w_qkv, w_proj):
    gum = _gumbel_full()
    eye = _eye_mask()
    wqt = np.ascontiguousarray(w_qkv.T).astype(np.float32).copy()
    wqt[:, :C] *= HD ** -0.5
    wpt = np.ascontiguousarray(w_proj.T).astype(np.float32)
    in_maps = []
    for i in range(NCORES):
        xs = np.ascontiguousarray(
            x[BPC * i:BPC * (i + 1)].reshape(TOK, C).T)
        gs = np.ascontiguousarray(gum[NSL * i:NSL * (i + 1)])
        in_maps.append({
            "xt": xs, "wqt": wqt, "wpt": wpt, "gum": gs, "eye": eye,
        })
    return in_maps


def kernel(x, w_qkv, w_proj, b_proj, _trace=False, _tracedir=None):
    x = np.asarray(x, np.float32)
    w_qkv = np.asarray(w_qkv, np.float32)
    w_proj = np.asarray(w_proj, np.float32)
    b_proj = np.asarray(b_proj, np.float32)
    nc = _build()
    in_maps = _prep_inputs(x, w_qkv, w_proj)
    kw = {}
    if _trace:
        kw = dict(trace=True, tmpdir=_tracedir)
    res = run_bass_kernel_spmd(nc, in_maps, core_ids=list(range(NCORES)), **kw)
    out = np.empty((B, N, C), np.float32)
    for i in range(NCORES):
        ot = np.asarray(res.results[i]["outt"])     # [C, TOK]
        out[BPC * i:BPC * (i + 1)] = ot.T.reshape(BPC, N, C)
    out += b_proj
    if _trace:
        return out, res
    return out


# revision 39
# speedup vs baseline: 1.3901x; 1.3901x over previous
"""Trainium2 Bass kernel for nn_Attention_84215718740239 (sparse attention
with Gumbel top-k mask dropout).

Strategy: data-parallel over batch (2 batches/core x 8 cores = 16 (b,h)
slices per core). All compute on-device except:
  - host-side layout prep (transposes of x / weights for DMA-friendly APs)
  - the Gumbel noise table, which is a data-independent constant
    (jax.random.key(42)) precomputed once on host
  - final un-shard (transpose back + bias add)

Device pipeline per core:
  B: qkv projection -> qT,kT (d-major, f32) and v (natural, bf16)
  C: per slice: attn logits (PE, f32) -> exp/stats/log-prob -> scores
     (scores = log(spatial)+log(aqm)+gumbel, diag pre-killed via the
     gumbel constant)
  D: per-slice top-k THRESHOLD via 6 count-sweeps (2 fixed probes +
     4 secant updates); counts by tensor_scalar(is_ge, accum_out) +
     PE ones-matmul partition reduction
  E: masked softmax applied multiplicatively (E2 = E * (score < thr)),
     DMA-transpose E2 (bf16), attn@v on PE, rows scaled by 1/rowsum
  F: output projection (bf16) producing out^T; host transposes back
"""

import numpy as np

import sys
if '/opt/trn_rl_repo' not in sys.path:
    sys.path.insert(0, '/opt/trn_rl_repo')

import concourse.bass as bass
import concourse.tile as tile
from concourse import bacc, mybir
from concourse.bass_utils import run_bass_kernel_spmd

F32 = mybir.dt.float32
BF16 = mybir.dt.bfloat16
AX = mybir.AxisListType
OP = mybir.AluOpType
AF = mybir.ActivationFunctionType

B, N, C, H, HD = 16, 512, 512, 8, 64
NCORES = 8
BPC = B // NCORES            # batches per core
NSL = BPC * H                # 16 slices per core
LF = N // 2                  # 256
KTARG = float(int(0.1 * (LF - 1)) * N)   # 12800 samples per slice
TOK = BPC * N                # 1024 tokens per core

_gum_cache = None


def _gumbel_full():
    """[128, 512, 256] f32 Gumbel noise (fixed key 42), diag pre-set to -1e30."""
    global _gum_cache
    if _gum_cache is None:
        import jax
        import jax.numpy as jnp
        with jax.default_device(jax.devices('cpu')[0]):
            u = jax.random.uniform(jax.random.key(42), (B * H, N * LF),
                                   dtype=jnp.float32, minval=1e-20, maxval=1.0)
            g = np.asarray(-jnp.log(-jnp.log(u))).reshape(B * H, N, LF).copy()
        g = np.exp(g)                       # exp-domain gumbel factor
        idx = np.arange(LF)
        g[:, idx, idx] = 0.0                # diagonal never sampled
        g[:, LF + idx, idx] = 0.0
        _gum_cache = g
    return _gum_cache


def _eye_mask():
    e = np.zeros((2, 128, LF), np.float32)
    p = np.arange(128)
    e[0, p, p] = 1.0
    e[1, p, 128 + p] = 1.0
    return e


_nc_cache = None
DEBUG_DUMP = False


def _build():
    global _nc_cache
    if _nc_cache is not None:
        return _nc_cache
    nc = bacc.Bacc("TRN2", target_bir_lowering=False, debug=False,
                   num_devices=NCORES)

    xt_d = nc.dram_tensor("xt", [C, TOK], F32, kind="ExternalInput").ap()
    wq_d = nc.dram_tensor("wqt", [C, 3 * C], F32, kind="ExternalInput").ap()
    wp_d = nc.dram_tensor("wpt", [C, C], F32, kind="ExternalInput").ap()
    gum_d = nc.dram_tensor("gum", [NSL, N, LF], F32, kind="ExternalInput").ap()
    eye_d = nc.dram_tensor("eye", [2, 128, LF], F32, kind="ExternalInput").ap()
    out_d = nc.dram_tensor("outt", [C, TOK], F32, kind="ExternalOutput").ap()

    dbg = None
    if DEBUG_DUMP:
        dbg = {}
        for name, shape in DEBUG_DUMP.items():
            dbg[name] = nc.dram_tensor(name, shape, F32,
                                       kind="ExternalOutput").ap()

    with tile.TileContext(nc) as tc:
        _emit(nc, tc, xt_d, wq_d, wp_d, gum_d, eye_d, out_d, dbg)
    nc.compile()
    _nc_cache = nc
    return nc


def _emit(nc, tc, xt_d, wq_d, wp_d, gum_d, eye_d, out_d, dbg=None):
    from contextlib import ExitStack
    ctx = ExitStack()
    with ctx:
        const = ctx.enter_context(tc.tile_pool(name="const", bufs=1))
        stats = ctx.enter_context(tc.tile_pool(name="stats", bufs=1))

        # ---------- load inputs ----------
        eye = const.tile([128, 2, LF], F32)
        nc.sync.dma_start(eye[:], eye_d.rearrange("e p c -> p e c"))

        ones_col = const.tile([128, 1], F32)      # lhsT for partition-sum
        nc.vector.memset(ones_col[:], 1.0)
        ones_row = const.tile([1, 128], F32)      # lhsT for partition-bcast
        nc.vector.memset(ones_row[:], 1.0)
        one16 = const.tile([1, NSL], F32)
        nc.vector.memset(one16[:], 1.0)
        c1em6 = const.tile([128, 1], F32)
        nc.vector.memset(c1em6[:], 1e-6)
        # count-conversion vectors: slices 0-7 counted on DVE (n = cnt),
        # slices 8-15 on ACT via Sign (n = 0.5*sgnsum + 65536)
        a16 = const.tile([1, NSL], F32)
        nc.vector.memset(a16[:, :8], 1.0)
        nc.vector.memset(a16[:, 8:], 0.5)
        b16 = const.tile([1, NSL], F32)
        nc.vector.memset(b16[:, :8], 0.0)
        nc.vector.memset(b16[:, 8:], float(N * LF / 2))

        wpb = const.tile([128, 4, C], BF16)
        vbf = const.tile([128, 8, C], BF16)       # v natural [tok, vc]
        so_all = stats.tile([128, NSL, 4], F32)   # opp-half exp row-sums
        thr = stats.tile([128, NSL], F32)         # bcast thresholds

        F32R = mybir.dt.float32r
        qkpool = ctx.enter_context(tc.tile_pool(name="qkp", bufs=1))
        qk = qkpool.tile([128, 8, TOK], F32R)     # q^T,k^T [outc, tok]

        # ---------- phase B: qkv projection ----------
        with tc.tile_pool(name="pbps", bufs=4, space="PSUM") as pb, \
             tc.tile_pool(name="pin", bufs=1) as pin:
            xt0 = pin.tile([128, 4, TOK], F32)    # x^T  [c, tok]
            nc.sync.dma_start(xt0[:],
                              xt_d.rearrange("(k p) t -> p k t", p=128))
            wq0 = pin.tile([128, 4, 3 * C], F32)  # w_qkv^T [c, outc]
            nc.sync.dma_start(wq0[:],
                              wq_d.rearrange("(k p) t -> p k t", p=128))
            wp = pin.tile([128, 4, C], F32)       # w_proj^T [c, oc]
            nc.sync.dma_start(wp[:], wp_d.rearrange("(k p) t -> p k t", p=128))
            for k in range(4):
                nc.vector.tensor_copy(wpb[:, k, :], wp[:, k, :])
            xt = pin.tile([128, 4, TOK], F32R)    # rounded for PE f32r
            wq = pin.tile([128, 4, 3 * C], F32R)
            for k in range(4):
                nc.vector.tensor_copy(xt[:, k, :], xt0[:, k, :])
                nc.scalar.copy(wq[:, k, :], wq0[:, k, :])
            for po in range(8):                   # outc tiles of q,k
                for tch in range(2):
                    ps = pb.tile([128, 512], F32)
                    for kc in range(4):
                        nc.tensor.matmul(
                            ps[:],
                            wq[:, kc, bass.ts(po, 128)],
                            xt[:, kc, bass.ts(tch, 512)],
                            start=(kc == 0), stop=(kc == 3))
                    if (po + tch) % 2:
                        nc.scalar.copy(qk[:, po, bass.ts(tch, 512)], ps[:])
                    else:
                        nc.vector.tensor_copy(qk[:, po, bass.ts(tch, 512)],
                                              ps[:])
            for tt in range(8):                   # v tok tiles
                ps = pb.tile([128, 512], F32)
                for kc in range(4):
                    nc.tensor.matmul(
                        ps[:],
                        xt[:, kc, bass.ts(tt, 128)],
                        wq[:, kc, bass.ds(2 * C, 512)],
                        start=(kc == 0), stop=(kc == 3))
                if tt % 2:
                    nc.scalar.copy(vbf[:, tt, :], ps[:])
                else:
                    nc.vector.tensor_copy(vbf[:, tt, :], ps[:])

        # persistent big tensors (allocated after phase-B scratch is freed)
        big = ctx.enter_context(tc.tile_pool(name="big", bufs=1))
        ebf = big.tile([128, NSL, 4, N], BF16)    # exp(attn) per slice/tile
        sc = big.tile([128, NSL, 4 * LF], F32)    # scores per slice

        # ---------- phase C: logits -> scores per slice ----------
        with tc.tile_pool(name="cps", bufs=4, space="PSUM") as cp, \
             tc.tile_pool(name="ef", bufs=2) as efp, \
             tc.tile_pool(name="gp", bufs=1) as gp, \
             tc.tile_pool(name="st4", bufs=4) as st4, \
             tc.tile_pool(name="gj", bufs=2) as gjp:
            for s in range(NSL):
                b, h = divmod(s, H)
                qpo, qpp = h // 2, 64 * (h % 2)
                gt = gp.tile([128, 4, LF], F32, tag="g")
                nc.sync.dma_start(
                    gt[:], gum_d[s].rearrange("(t p) c -> p t c", p=128))
                ssr4 = st4.tile([128, 4], F32, tag="ssr")
                so4v = so_all[:, s, :]
                eii4 = st4.tile([128, 4], F32, tag="eii")
                mx4 = st4.tile([128, 4], F32, tag="mx")
                ef = efp.tile([128, 4, 512], F32, tag="ef")
                for t in range(4):
                    soff = 0 if t < 2 else LF
                    ooff = LF - soff
                    ps = cp.tile([128, 512], F32, tag="attn")
                    lhs = qk[qpp:qpp + 64, qpo, bass.ds(b * N + t * 128, 128)]
                    rhs = qk[qpp:qpp + 64, 4 + qpo, bass.ds(b * N, 512)]
                    nc.tensor.matmul(ps[:], lhs, rhs, start=True, stop=True)
                    # row max of opposite block (logit domain, f32)
                    nc.vector.tensor_reduce(mx4[:, t:t + 1],
                                            ps[:, ooff:ooff + LF],
                                            axis=AX.X, op=OP.max)
                    nc.scalar.activation(ef[:, t, soff:soff + LF],
                                         ps[:, soff:soff + LF], AF.Exp,
                                         accum_out=ssr4[:, t:t + 1])
                    nc.scalar.activation(ef[:, t, ooff:ooff + LF],
                                         ps[:, ooff:ooff + LF], AF.Exp,
                                         accum_out=so4v[:, t:t + 1])
                    # diagonal element of the same-block
                    gj = gjp.tile([128, LF], F32, tag="gj")
                    nc.vector.scalar_tensor_tensor(
                        gj[:], ef[:, t, soff:soff + LF], 1.0, eye[:, t % 2, :],
                        op0=OP.mult, op1=OP.mult,
                        accum_out=eii4[:, t:t + 1])
                nc.vector.tensor_copy(ebf[:, s, :, :], ef[:])
                # per-row scalars for all 4 tiles at once  [128,4]
                d4 = st4.tile([128, 4], F32, tag="d4")
                nc.vector.tensor_tensor(d4[:], ssr4[:], so4v, op=OP.add)
                rd4 = st4.tile([128, 4], F32, tag="rd4")
                nc.vector.reciprocal(rd4[:], d4[:])
                sv4 = st4.tile([128, 4], F32, tag="sv4")
                nc.vector.tensor_tensor(sv4[:], ssr4[:], eii4[:],
                                        op=OP.subtract)
                nc.vector.tensor_tensor(sv4[:], sv4[:], rd4[:], op=OP.mult)
                nc.vector.tensor_scalar(sv4[:], sv4[:], float((LF - 1) * 1e-6),
                                        None, op0=OP.add)
                aqm4 = st4.tile([128, 4], F32, tag="aqm")
                nc.scalar.activation(aqm4[:], mx4[:], AF.Exp)
                nc.vector.tensor_tensor(aqm4[:], aqm4[:], rd4[:], op=OP.mult)
                nc.vector.tensor_scalar(aqm4[:], aqm4[:], 1e-6, None,
                                        op0=OP.add)
                # rq = aqm / S  (exp-domain row factor; no logs anywhere)
                rq4 = st4.tile([128, 4], F32, tag="rq4")
                nc.vector.reciprocal(rq4[:], sv4[:])
                nc.vector.tensor_tensor(rq4[:], rq4[:], aqm4[:], op=OP.mult)
                for t in range(4):
                    soff = 0 if t < 2 else LF
                    slot = sc[:, s, bass.ts(t, LF)]
                    # aw' = E*recipD + 1e-6 ; zsc = aw' * rq * exp(g)
                    aw1 = gjp.tile([128, LF], F32, tag="gj")
                    nc.scalar.activation(aw1[:], ef[:, t, soff:soff + LF],
                                         AF.Identity, scale=rd4[:, t:t + 1],
                                         bias=c1em6[:])
                    nc.vector.scalar_tensor_tensor(
                        slot, aw1[:], rq4[:, t:t + 1], gt[:, t, :],
                        op0=OP.mult, op1=OP.mult)

        # ---------- phase D: threshold selection (secant, 6 sweeps) ----------
        with tc.tile_pool(name="dps", bufs=2, space="PSUM") as dps, \
             tc.tile_pool(name="dbc", bufs=2, space="PSUM") as dbc, \
             tc.tile_pool(name="sel", bufs=1) as sel, \
             tc.tile_pool(name="junk", bufs=2) as jp, \
             tc.tile_pool(name="s16", bufs=8) as s16:
            import math
            cnt128 = sel.tile([128, NSL], F32)
            thrN = sel.tile([128, NSL], F32)
            t0 = sel.tile([1, NSL], F32)
            t1 = sel.tile([1, NSL], F32)
            tneg = sel.tile([1, NSL], F32)
            n0 = sel.tile([1, NSL], F32)
            n1 = sel.tile([1, NSL], F32)
            nc.vector.memset(t0[:], math.exp(-9.10))
            nc.vector.memset(t1[:], math.exp(-8.80))

            def count_into(ndst):
                for s in range(8):
                    jk = jp.tile([128, 4 * LF], BF16, tag="junk")
                    nc.vector.tensor_scalar(jk[:], sc[:, s, :],
                                            thr[:, s:s + 1], 0.0,
                                            op0=OP.is_ge, op1=OP.add,
                                            accum_out=cnt128[:, s:s + 1])
                for s in range(8, NSL):
                    jk2 = jp.tile([128, 4 * LF], BF16, tag="junk2")
                    nc.scalar.activation(jk2[:], sc[:, s, :], AF.Sign,
                                         bias=thrN[:, s:s + 1],
                                         accum_out=cnt128[:, s:s + 1])
                cp_ = dps.tile([1, NSL], F32, tag="cnt")
                nc.tensor.matmul(cp_[:], ones_col[:], cnt128[:],
                                 start=True, stop=True)
                nc.vector.tensor_tensor(ndst, cp_[:], a16[:], op=OP.mult)
                nc.vector.tensor_tensor(ndst, ndst, b16[:], op=OP.add)

            def bcast(tsrc):
                bp = dbc.tile([128, NSL], F32, tag="bc")
                nc.tensor.matmul(bp[:], ones_row[:], tsrc, start=True,
                                 stop=True)
                nc.vector.tensor_copy(thr[:], bp[:])
                nc.vector.tensor_scalar(tneg[:], tsrc, -1.0, None,
                                        op0=OP.mult)
                bp2 = dbc.tile([128, NSL], F32, tag="bc2")
                nc.tensor.matmul(bp2[:], ones_row[:], tneg[:], start=True,
                                 stop=True)
                nc.vector.tensor_copy(thrN[:], bp2[:])

            bcast(t0[:])
            count_into(n0[:])
            bcast(t1[:])
            count_into(n1[:])
            for r in range(4):
                d = s16.tile([1, NSL], F32, tag="d")
                nc.vector.tensor_tensor(d[:], n1[:], n0[:], op=OP.subtract)
                dsq = s16.tile([1, NSL], F32, tag="dsq")
                nc.vector.tensor_tensor(dsq[:], d[:], d[:], op=OP.mult)
                msk = s16.tile([1, NSL], F32, tag="msk")
                nc.vector.tensor_scalar(msk[:], dsq[:], 1.0, None,
                                        op0=OP.is_lt)
                nc.vector.tensor_tensor(d[:], d[:], msk[:], op=OP.subtract)
                rcd = s16.tile([1, NSL], F32, tag="rcd")
                nc.vector.reciprocal(rcd[:], d[:])
                dt = s16.tile([1, NSL], F32, tag="dt")
                nc.vector.tensor_tensor(dt[:], t1[:], t0[:], op=OP.subtract)
                nk = s16.tile([1, NSL], F32, tag="nk")
                nc.vector.tensor_scalar(nk[:], n1[:], KTARG, None,
                                        op0=OP.subtract)
                nc.vector.tensor_tensor(nk[:], nk[:], dt[:], op=OP.mult)
                nc.vector.tensor_tensor(nk[:], nk[:], rcd[:], op=OP.mult)
                t2 = s16.tile([1, NSL], F32, tag="t2")
                nc.vector.tensor_tensor(t2[:], t1[:], nk[:], op=OP.subtract)
                nc.vector.tensor_scalar(t2[:], t2[:], math.exp(-8.0),
                                        math.exp(-10.0),
                                        op0=OP.min, op1=OP.max)
                nc.vector.tensor_copy(t0[:], t1[:])
                nc.vector.tensor_copy(n0[:], n1[:])
                nc.vector.tensor_copy(t1[:], t2[:])
                bcast(t1[:])
                count_into(n1[:])
            if dbg is not None and "dbg_n" in dbg:
                nn = sel.tile([1, 2 * NSL], F32)
                nc.vector.tensor_copy(nn[:, :NSL], n0[:])
                nc.vector.tensor_copy(nn[:, NSL:], n1[:])
                nc.sync.dma_start(dbg["dbg_n"], nn[:])

        if dbg is not None:
            srcs = {"dbg_sc": sc, "dbg_thr": thr, "dbg_so": so_all,
                    "dbg_qk": qk}
            for name, t_ in srcs.items():
                if name in dbg:
                    nc.sync.dma_start(dbg[name], t_[:].bitcast(F32))

        # ---------- phase E: masked softmax + attn@v ----------
        obf = big.tile([128, 8, C], BF16)         # attn-out natural [tok, c]
        with tc.tile_pool(name="eps", bufs=4, space="PSUM") as ep, \
             tc.tile_pool(name="e2p", bufs=2) as e2p, \
             tc.tile_pool(name="e2tp", bufs=2) as e2tp, \
             tc.tile_pool(name="est", bufs=2) as est:
            for s in range(NSL):
                b, h = divmod(s, H)
                e2 = e2p.tile([128, 4, N], BF16, tag="e2")
                ss2 = est.tile([128, 4], F32, tag="ss2")
                for t in range(4):
                    soff = 0 if t < 2 else LF
                    ooff = LF - soff
                    nc.vector.scalar_tensor_tensor(
                        e2[:, t, soff:soff + LF], sc[:, s, bass.ts(t, LF)],
                        thr[:, s:s + 1], ebf[:, s, t, soff:soff + LF],
                        op0=OP.is_lt, op1=OP.mult,
                        accum_out=ss2[:, t:t + 1])
                    nc.vector.tensor_copy(e2[:, t, ooff:ooff + LF],
                                          ebf[:, s, t, ooff:ooff + LF])
                rd2 = est.tile([128, 4], F32, tag="rd2")
                nc.vector.tensor_tensor(rd2[:], ss2[:], so_all[:, s, :],
                                        op=OP.add)
                nc.vector.reciprocal(rd2[:], rd2[:])
                e2t = e2tp.tile([128, 4, N], BF16, tag="e2t")
                for u in range(4):
                    for t in range(4):
                        nc.sync.dma_start_transpose(
                            e2t[:, u, bass.ts(t, 128)],
                            e2[:, t, bass.ts(u, 128)])
                for t in range(4):
                    ps = ep.tile([128, 64], F32, tag="av")
                    for u in range(4):
                        nc.tensor.matmul(ps[:],
                                         e2t[:, u, bass.ts(t, 128)],
                                         vbf[:, 4 * b + u, bass.ts(h, 64)],
                                         start=(u == 0), stop=(u == 3))
                    nc.vector.tensor_scalar(obf[:, 4 * b + t, bass.ts(h, 64)],
                                            ps[:], rd2[:, t:t + 1], None,
                                            op0=OP.mult)

        # ---------- phase F: output projection ----------
        with tc.tile_pool(name="fps", bufs=4, space="PSUM") as fp, \
             tc.tile_pool(name="obt", bufs=1) as obtp, \
             tc.tile_pool(name="o2", bufs=1) as o2p:
            obt = obtp.tile([128, 4, TOK], BF16)  # out^T [c, tok]
            for ct in range(4):
                for tt in range(8):
                    nc.sync.dma_start_transpose(obt[:, ct, bass.ts(tt, 128)],
                                            obf[:, tt, bass.ts(ct, 128)])
            o2 = o2p.tile([128, 4, TOK], F32)     # proj out^T [oc, tok]
            for oc in range(4):
                for tch in range(2):
                    ps = fp.tile([128, 512], F32, tag="pj")
                    for ct in range(4):
                        nc.tensor.matmul(ps[:], wpb[:, ct, bass.ts(oc, 128)],
                                         obt[:, ct, bass.ts(tch, 512)],
                                         start=(ct == 0), stop=(ct == 3))
                    if (oc + tch) % 2:
                        nc.scalar.copy(o2[:, oc, bass.ts(tch, 512)], ps[:])
                    else:
                        nc.vector.tensor_copy(o2[:, oc, bass.ts(tch, 512)],
                                              ps[:])
            nc.sync.dma_start(out_d.rearrange("(k p) t -> p k t", p=128),
                              o2[:])


def _prep_inputs(x, w_qkv, w_proj):
    gum = _gumbel_full()
    eye = _eye_mask()
    wqt = np.ascontiguousarray(w_qkv.T).astype(np.float32).copy()
    wqt[:, :C] *= HD ** -0.5
    wpt = np.ascontiguousarray(w_proj.T).astype(np.float32)
    in_maps = []
    for i in range(NCORES):
        xs = np.ascontiguousarray(
            x[BPC * i:BPC * (i + 1)].reshape(TOK, C).T)
        gs = np.ascontiguousarray(gum[NSL * i:NSL * (i + 1)])
        in_maps.append({
            "xt": xs, "wqt": wqt, "wpt": wpt, "gum": gs, "eye": eye,
        })
    return in_maps


def kernel(x, w_qkv, w_proj, b_proj, _trace=False, _tracedir=None):
    x = np.asarray(x, np.float32)
    w_qkv = np.asarray(w_qkv, np.float32)
    w_proj = np.asarray(w_proj, np.float32)
    b_proj = np.asarray(b_proj, np.float32)
    nc = _build()
    in_maps = _prep_inputs(x, w_qkv, w_proj)
    kw = {}
    if _trace:
        kw = dict(trace=True, tmpdir=_tracedir)
    res = run_bass_kernel_spmd(nc, in_maps, core_ids=list(range(NCORES)), **kw)
    out = np.empty((B, N, C), np.float32)
    for i in range(NCORES):
        ot = np.asarray(res.results[i]["outt"])     # [C, TOK]
        out[BPC * i:BPC * (i + 1)] = ot.T.reshape(BPC, N, C)
    out += b_proj
    if _trace:
        return out, res
    return out


# revision 42
# speedup vs baseline: 2.0054x; 1.4426x over previous
"""Trainium2 Bass kernel for nn_Attention_84215718740239 (sparse attention
with Gumbel top-k mask dropout).

Strategy: data-parallel over batch (2 batches/core x 8 cores = 16 (b,h)
slices per core). All compute on-device except:
  - host-side layout prep (transposes of x / weights for DMA-friendly APs)
  - the Gumbel noise table, which is a data-independent constant
    (jax.random.key(42)) precomputed once on host
  - final un-shard (transpose back + bias add)

Device pipeline per core:
  B: qkv projection -> qT,kT (d-major, f32) and v (natural, bf16)
  C: per slice: attn logits (PE, f32) -> exp/stats/log-prob -> scores
     (scores = log(spatial)+log(aqm)+gumbel, diag pre-killed via the
     gumbel constant)
  D: per-slice top-k THRESHOLD via 6 count-sweeps (2 fixed probes +
     4 secant updates); counts by tensor_scalar(is_ge, accum_out) +
     PE ones-matmul partition reduction
  E: masked softmax applied multiplicatively (E2 = E * (score < thr)),
     DMA-transpose E2 (bf16), attn@v on PE, rows scaled by 1/rowsum
  F: output projection (bf16) producing out^T; host transposes back
"""

import numpy as np

import sys
if '/opt/trn_rl_repo' not in sys.path:
    sys.path.insert(0, '/opt/trn_rl_repo')

import concourse.bass as bass
import concourse.tile as tile
from concourse import bacc, mybir
from concourse.bass_utils import run_bass_kernel_spmd

F32 = mybir.dt.float32
BF16 = mybir.dt.bfloat16
AX = mybir.AxisListType
OP = mybir.AluOpType
AF = mybir.ActivationFunctionType

B, N, C, H, HD = 16, 512, 512, 8, 64
NCORES = 8
BPC = B // NCORES            # batches per core
NSL = BPC * H                # 16 slices per core
LF = N // 2                  # 256
KTARG = float(int(0.1 * (LF - 1)) * N)   # 12800 samples per slice
TOK = BPC * N                # 1024 tokens per core

_gum_cache = None


def _gumbel_full():
    """[128, 512, 256] f32 Gumbel noise (fixed key 42), diag pre-set to -1e30."""
    global _gum_cache
    if _gum_cache is None:
        import jax
        import jax.numpy as jnp
        with jax.default_device(jax.devices('cpu')[0]):
            u = jax.random.uniform(jax.random.key(42), (B * H, N * LF),
                                   dtype=jnp.float32, minval=1e-20, maxval=1.0)
            g = np.asarray(-jnp.log(-jnp.log(u))).reshape(B * H, N, LF).copy()
        g = np.exp(g)                       # exp-domain gumbel factor
        idx = np.arange(LF)
        g[:, idx, idx] = 0.0                # diagonal never sampled
        g[:, LF + idx, idx] = 0.0
        _gum_cache = g
    return _gum_cache


def _eye_mask():
    e = np.zeros((2, 128, LF), np.float32)
    p = np.arange(128)
    e[0, p, p] = 1.0
    e[1, p, 128 + p] = 1.0
    return e


_nc_cache = None
DEBUG_DUMP = False


def _build():
    global _nc_cache
    if _nc_cache is not None:
        return _nc_cache
    nc = bacc.Bacc("TRN2", target_bir_lowering=False, debug=False,
                   num_devices=NCORES)

    xt_d = nc.dram_tensor("xt", [C, TOK], F32, kind="ExternalInput").ap()
    wq_d = nc.dram_tensor("wqt", [C, 3 * C], F32, kind="ExternalInput").ap()
    wp_d = nc.dram_tensor("wpt", [C, C], F32, kind="ExternalInput").ap()
    gum_d = nc.dram_tensor("gum", [NSL, N, LF], F32, kind="ExternalInput").ap()
    eye_d = nc.dram_tensor("eye", [2, 128, LF], F32, kind="ExternalInput").ap()
    out_d = nc.dram_tensor("outt", [C, TOK], F32, kind="ExternalOutput").ap()

    dbg = None
    if DEBUG_DUMP:
        dbg = {}
        for name, shape in DEBUG_DUMP.items():
            dbg[name] = nc.dram_tensor(name, shape, F32,
                                       kind="ExternalOutput").ap()

    with tile.TileContext(nc) as tc:
        _emit(nc, tc, xt_d, wq_d, wp_d, gum_d, eye_d, out_d, dbg)
    nc.compile()
    _nc_cache = nc
    return nc


def _emit(nc, tc, xt_d, wq_d, wp_d, gum_d, eye_d, out_d, dbg=None):
    from contextlib import ExitStack
    ctx = ExitStack()
    with ctx:
        const = ctx.enter_context(tc.tile_pool(name="const", bufs=1))
        stats = ctx.enter_context(tc.tile_pool(name="stats", bufs=1))

        # ---------- load inputs ----------
        eye = const.tile([128, 2, LF], F32)
        nc.sync.dma_start(eye[:], eye_d.rearrange("e p c -> p e c"))

        ones_col = const.tile([128, 1], F32)      # lhsT for partition-sum
        nc.vector.memset(ones_col[:], 1.0)
        ones_row = const.tile([1, 128], F32)      # lhsT for partition-bcast
        nc.vector.memset(ones_row[:], 1.0)
        one16 = const.tile([1, NSL], F32)
        nc.vector.memset(one16[:], 1.0)
        ones64 = const.tile([1, 64], F32)
        nc.vector.memset(ones64[:], 1.0)
        ones_col_bf = const.tile([128, 1], BF16)
        nc.vector.memset(ones_col_bf[:], 1.0)
        idbf = const.tile([128, 128], BF16)
        from concourse import masks as _masks
        _masks.make_identity(nc, idbf[:])
        c1em6 = const.tile([128, 1], F32)
        nc.vector.memset(c1em6[:], 1e-6)
        # count-conversion vectors: slices 0-7 counted on DVE (n = cnt),
        # slices 8-15 on ACT via Sign (n = 0.5*sgnsum + 65536)
        a16 = const.tile([1, NSL], F32)
        nc.vector.memset(a16[:, :8], 1.0)
        nc.vector.memset(a16[:, 8:], 0.5)
        b16 = const.tile([1, NSL], F32)
        nc.vector.memset(b16[:, :8], 0.0)
        nc.vector.memset(b16[:, 8:], float(N * LF / 2))

        wpb = const.tile([128, 4, C], BF16)
        vbf = const.tile([128, 8, C], BF16)       # v natural [tok, vc]
        so_all = stats.tile([128, NSL, 4], F32)   # opp-half exp row-sums
        thr = stats.tile([128, NSL], F32)         # bcast thresholds

        F32R = mybir.dt.float32r
        qkpool = ctx.enter_context(tc.tile_pool(name="qkp", bufs=1))
        qk = qkpool.tile([128, 8, TOK], F32R)     # q^T,k^T [outc, tok]

        # ---------- phase B: qkv projection ----------
        with tc.tile_pool(name="pbps", bufs=4, space="PSUM") as pb, \
             tc.tile_pool(name="pin", bufs=1) as pin:
            xt0 = pin.tile([128, 4, TOK], F32)    # x^T  [c, tok]
            nc.sync.dma_start(xt0[:],
                              xt_d.rearrange("(k p) t -> p k t", p=128))
            wq0 = pin.tile([128, 4, 3 * C], F32)  # w_qkv^T [c, outc]
            nc.sync.dma_start(wq0[:],
                              wq_d.rearrange("(k p) t -> p k t", p=128))
            wp = pin.tile([128, 4, C], F32)       # w_proj^T [c, oc]
            nc.sync.dma_start(wp[:], wp_d.rearrange("(k p) t -> p k t", p=128))
            for k in range(4):
                nc.vector.tensor_copy(wpb[:, k, :], wp[:, k, :])
            xt = pin.tile([128, 4, TOK], F32R)    # rounded for PE f32r
            wq = pin.tile([128, 4, 3 * C], F32R)
            for k in range(4):
                nc.vector.tensor_copy(xt[:, k, :], xt0[:, k, :])
                nc.scalar.copy(wq[:, k, :], wq0[:, k, :])
            for po in range(8):                   # outc tiles of q,k
                for tch in range(2):
                    ps = pb.tile([128, 512], F32)
                    for kc in range(4):
                        nc.tensor.matmul(
                            ps[:],
                            wq[:, kc, bass.ts(po, 128)],
                            xt[:, kc, bass.ts(tch, 512)],
                            start=(kc == 0), stop=(kc == 3))
                    if (po + tch) % 2:
                        nc.scalar.copy(qk[:, po, bass.ts(tch, 512)], ps[:])
                    else:
                        nc.vector.tensor_copy(qk[:, po, bass.ts(tch, 512)],
                                              ps[:])
            for tt in range(8):                   # v tok tiles
                ps = pb.tile([128, 512], F32)
                for kc in range(4):
                    nc.tensor.matmul(
                        ps[:],
                        xt[:, kc, bass.ts(tt, 128)],
                        wq[:, kc, bass.ds(2 * C, 512)],
                        start=(kc == 0), stop=(kc == 3))
                if tt % 2:
                    nc.scalar.copy(vbf[:, tt, :], ps[:])
                else:
                    nc.vector.tensor_copy(vbf[:, tt, :], ps[:])

        # persistent big tensors (allocated after phase-B scratch is freed)
        big = ctx.enter_context(tc.tile_pool(name="big", bufs=1))
        sc = big.tile([128, NSL, 4 * LF], F32)    # scores per slice

        # ---------- phase C: logits -> scores per slice ----------
        with tc.tile_pool(name="cps", bufs=4, space="PSUM") as cp, \
             tc.tile_pool(name="ef", bufs=2) as efp, \
             tc.tile_pool(name="gp", bufs=1) as gp, \
             tc.tile_pool(name="st4", bufs=4) as st4, \
             tc.tile_pool(name="gj", bufs=2) as gjp:
            for s in range(NSL):
                b, h = divmod(s, H)
                qpo, qpp = h // 2, 64 * (h % 2)
                gt = gp.tile([128, 4, LF], F32, tag="g")
                nc.sync.dma_start(
                    gt[:], gum_d[s].rearrange("(t p) c -> p t c", p=128))
                ssr4 = st4.tile([128, 4], F32, tag="ssr")
                so4v = so_all[:, s, :]
                eii4 = st4.tile([128, 4], F32, tag="eii")
                mx4 = st4.tile([128, 4], F32, tag="mx")
                ef = efp.tile([128, 4, 512], F32, tag="ef")
                for t in range(4):
                    soff = 0 if t < 2 else LF
                    ooff = LF - soff
                    ps = cp.tile([128, 512], F32, tag="attn")
                    lhs = qk[qpp:qpp + 64, qpo, bass.ds(b * N + t * 128, 128)]
                    rhs = qk[qpp:qpp + 64, 4 + qpo, bass.ds(b * N, 512)]
                    nc.tensor.matmul(ps[:], lhs, rhs, start=True, stop=True)
                    # row max of opposite block (logit domain, f32)
                    nc.vector.tensor_reduce(mx4[:, t:t + 1],
                                            ps[:, ooff:ooff + LF],
                                            axis=AX.X, op=OP.max)
                    nc.scalar.activation(ef[:, t, soff:soff + LF],
                                         ps[:, soff:soff + LF], AF.Exp,
                                         accum_out=ssr4[:, t:t + 1])
                    nc.scalar.activation(ef[:, t, ooff:ooff + LF],
                                         ps[:, ooff:ooff + LF], AF.Exp,
                                         accum_out=so4v[:, t:t + 1])
                    # diagonal element of the same-block
                    gj = gjp.tile([128, LF], F32, tag="gj")
                    nc.vector.scalar_tensor_tensor(
                        gj[:], ef[:, t, soff:soff + LF], 1.0, eye[:, t % 2, :],
                        op0=OP.mult, op1=OP.mult,
                        accum_out=eii4[:, t:t + 1])
                # per-row scalars for all 4 tiles at once  [128,4]
                d4 = st4.tile([128, 4], F32, tag="d4")
                nc.vector.tensor_tensor(d4[:], ssr4[:], so4v, op=OP.add)
                rd4 = st4.tile([128, 4], F32, tag="rd4")
                nc.vector.reciprocal(rd4[:], d4[:])
                sv4 = st4.tile([128, 4], F32, tag="sv4")
                nc.vector.tensor_tensor(sv4[:], ssr4[:], eii4[:],
                                        op=OP.subtract)
                nc.vector.tensor_tensor(sv4[:], sv4[:], rd4[:], op=OP.mult)
                nc.vector.tensor_scalar(sv4[:], sv4[:], float((LF - 1) * 1e-6),
                                        None, op0=OP.add)
                aqm4 = st4.tile([128, 4], F32, tag="aqm")
                nc.scalar.activation(aqm4[:], mx4[:], AF.Exp)
                nc.vector.tensor_tensor(aqm4[:], aqm4[:], rd4[:], op=OP.mult)
                nc.vector.tensor_scalar(aqm4[:], aqm4[:], 1e-6, None,
                                        op0=OP.add)
                # rq = aqm / S  (exp-domain row factor; no logs anywhere)
                rq4 = st4.tile([128, 4], F32, tag="rq4")
                nc.vector.reciprocal(rq4[:], sv4[:])
                nc.vector.tensor_tensor(rq4[:], rq4[:], aqm4[:], op=OP.mult)
                for t in range(4):
                    soff = 0 if t < 2 else LF
                    slot = sc[:, s, bass.ts(t, LF)]
                    # aw' = E*recipD + 1e-6 ; zsc = aw' * rq * exp(g)
                    aw1 = gjp.tile([128, LF], F32, tag="gj")
                    nc.scalar.activation(aw1[:], ef[:, t, soff:soff + LF],
                                         AF.Identity, scale=rd4[:, t:t + 1],
                                         bias=c1em6[:])
                    nc.vector.scalar_tensor_tensor(
                        slot, aw1[:], rq4[:, t:t + 1], gt[:, t, :],
                        op0=OP.mult, op1=OP.mult)

        # ---------- phase D: threshold selection (secant, 6 sweeps) ----------
        with tc.tile_pool(name="dps", bufs=2, space="PSUM") as dps, \
             tc.tile_pool(name="dbc", bufs=2, space="PSUM") as dbc, \
             tc.tile_pool(name="sel", bufs=1) as sel, \
             tc.tile_pool(name="junk", bufs=2) as jp, \
             tc.tile_pool(name="s16", bufs=8) as s16:
            import math
            cnt128 = sel.tile([128, NSL], F32)
            thrN = sel.tile([128, NSL], F32)
            t0 = sel.tile([1, NSL], F32)
            t1 = sel.tile([1, NSL], F32)
            tneg = sel.tile([1, NSL], F32)
            n0 = sel.tile([1, NSL], F32)
            n1 = sel.tile([1, NSL], F32)
            nc.vector.memset(t0[:], math.exp(-9.10))
            nc.vector.memset(t1[:], math.exp(-8.80))

            def count_into(ndst):
                for s in range(8):
                    jk = jp.tile([128, 4 * LF], BF16, tag="junk")
                    nc.vector.tensor_scalar(jk[:], sc[:, s, :],
                                            thr[:, s:s + 1], 0.0,
                                            op0=OP.is_ge, op1=OP.add,
                                            accum_out=cnt128[:, s:s + 1])
                for s in range(8, NSL):
                    jk2 = jp.tile([128, 4 * LF], BF16, tag="junk2")
                    nc.scalar.activation(jk2[:], sc[:, s, :], AF.Sign,
                                         bias=thrN[:, s:s + 1],
                                         accum_out=cnt128[:, s:s + 1])
                cp_ = dps.tile([1, NSL], F32, tag="cnt")
                nc.tensor.matmul(cp_[:], ones_col[:], cnt128[:],
                                 start=True, stop=True)
                nc.vector.tensor_tensor(ndst, cp_[:], a16[:], op=OP.mult)
                nc.vector.tensor_tensor(ndst, ndst, b16[:], op=OP.add)

            def bcast(tsrc):
                bp = dbc.tile([128, NSL], F32, tag="bc")
                nc.tensor.matmul(bp[:], ones_row[:], tsrc, start=True,
                                 stop=True)
                nc.vector.tensor_copy(thr[:], bp[:])
                nc.vector.tensor_scalar(tneg[:], tsrc, -1.0, None,
                                        op0=OP.mult)
                bp2 = dbc.tile([128, NSL], F32, tag="bc2")
                nc.tensor.matmul(bp2[:], ones_row[:], tneg[:], start=True,
                                 stop=True)
                nc.vector.tensor_copy(thrN[:], bp2[:])

            bcast(t0[:])
            count_into(n0[:])
            bcast(t1[:])
            count_into(n1[:])
            for r in range(4):
                d = s16.tile([1, NSL], F32, tag="d")
                nc.vector.tensor_tensor(d[:], n1[:], n0[:], op=OP.subtract)
                dsq = s16.tile([1, NSL], F32, tag="dsq")
                nc.vector.tensor_tensor(dsq[:], d[:], d[:], op=OP.mult)
                msk = s16.tile([1, NSL], F32, tag="msk")
                nc.vector.tensor_scalar(msk[:], dsq[:], 1.0, None,
                                        op0=OP.is_lt)
                nc.vector.tensor_tensor(d[:], d[:], msk[:], op=OP.subtract)
                rcd = s16.tile([1, NSL], F32, tag="rcd")
                nc.vector.reciprocal(rcd[:], d[:])
                dt = s16.tile([1, NSL], F32, tag="dt")
                nc.vector.tensor_tensor(dt[:], t1[:], t0[:], op=OP.subtract)
                nk = s16.tile([1, NSL], F32, tag="nk")
                nc.vector.tensor_scalar(nk[:], n1[:], KTARG, None,
                                        op0=OP.subtract)
                nc.vector.tensor_tensor(nk[:], nk[:], dt[:], op=OP.mult)
                nc.vector.tensor_tensor(nk[:], nk[:], rcd[:], op=OP.mult)
                t2 = s16.tile([1, NSL], F32, tag="t2")
                nc.vector.tensor_tensor(t2[:], t1[:], nk[:], op=OP.subtract)
                nc.vector.tensor_scalar(t2[:], t2[:], math.exp(-8.0),
                                        math.exp(-10.0),
                                        op0=OP.min, op1=OP.max)
                nc.vector.tensor_copy(t0[:], t1[:])
                nc.vector.tensor_copy(n0[:], n1[:])
                nc.vector.tensor_copy(t1[:], t2[:])
                bcast(t1[:])
                count_into(n1[:])
            if dbg is not None and "dbg_n" in dbg:
                nn = sel.tile([1, 2 * NSL], F32)
                nc.vector.tensor_copy(nn[:, :NSL], n0[:])
                nc.vector.tensor_copy(nn[:, NSL:], n1[:])
                nc.sync.dma_start(dbg["dbg_n"], nn[:])

        if dbg is not None:
            srcs = {"dbg_sc": sc, "dbg_thr": thr, "dbg_so": so_all,
                    "dbg_qk": qk}
            for name, t_ in srcs.items():
                if name in dbg:
                    nc.sync.dma_start(dbg[name], t_[:].bitcast(F32))

        # ---------- phase E: masked softmax + attn@v (transposed) ----------
        obtp = ctx.enter_context(tc.tile_pool(name="obt", bufs=1))
        obt = obtp.tile([128, 4, TOK], BF16)      # attn-out^T [c, tok]
        with tc.tile_pool(name="eaT", bufs=2, space="PSUM") as eaT, \
             tc.tile_pool(name="ekt", bufs=2, space="PSUM") as ektp, \
             tc.tile_pool(name="ed2", bufs=1, space="PSUM") as ed2p, \
             tc.tile_pool(name="erb", bufs=1, space="PSUM") as erbp, \
             tc.tile_pool(name="epo", bufs=2, space="PSUM") as epop, \
             tc.tile_pool(name="e2tp", bufs=2) as e2tp, \
             tc.tile_pool(name="ek01", bufs=2) as k01p, \
             tc.tile_pool(name="est", bufs=2) as est:
            for s in range(NSL):
                b, h = divmod(s, H)
                qpo, qpp = h // 2, 64 * (h % 2)
                e2t = e2tp.tile([128, 4, N], BF16, tag="e2t")
                # attn^T per kcol-tile u  ->  E^T = exp (bf16)
                for u in range(4):
                    psT = eaT.tile([128, 512], F32, tag="aT")
                    lhs = qk[qpp:qpp + 64, 4 + qpo,
                             bass.ds(b * N + u * 128, 128)]
                    rhs = qk[qpp:qpp + 64, qpo, bass.ds(b * N, 512)]
                    nc.tensor.matmul(psT[:], lhs, rhs, start=True, stop=True)
                    nc.scalar.activation(e2t[:, u, :], psT[:], AF.Exp)
                # keep mask (natural), 0/1 bf16
                k01 = k01p.tile([128, 4, LF], BF16, tag="k01")
                for t in range(4):
                    nc.vector.tensor_scalar(k01[:, t, :],
                                            sc[:, s, bass.ts(t, LF)],
                                            thr[:, s:s + 1], None,
                                            op0=OP.is_lt)
                # transpose keep blocks on PE, apply to diag quadrants
                for t in range(4):
                    qoff = 0 if t < 2 else LF     # qrow quadrant offset
                    for uu in range(2):
                        u = 2 * (t // 2) + uu
                        ktp = ektp.tile([128, 128], BF16, tag="ktp")
                        nc.tensor.transpose(ktp[:],
                                            k01[:, t, bass.ts(uu, 128)],
                                            idbf[:])
                        dst = e2t[:, u, qoff + (t % 2) * 128:
                                  qoff + (t % 2) * 128 + 128]
                        nc.vector.tensor_tensor(dst, dst, ktp[:], op=OP.mult)
                # D2 row-sums via ones-matmul over kcol partitions
                d2 = ed2p.tile([1, N], F32, tag="d2")
                for u in range(4):
                    nc.tensor.matmul(d2[:], ones_col_bf[:], e2t[:, u, :],
                                     start=(u == 0), stop=(u == 3))
                rd2 = est.tile([1, N], F32, tag="rd2")
                nc.vector.reciprocal(rd2[:], d2[:])
                rdb = erbp.tile([64, N], F32, tag="rdb")
                nc.tensor.matmul(rdb[:], ones64[:], rd2[:],
                                 start=True, stop=True)
                # out^T = v^T @ p^T  (accumulate over kcol chunks)
                po = epop.tile([64, N], F32, tag="po")
                for u in range(4):
                    nc.tensor.matmul(po[:],
                                     vbf[:, 4 * b + u, bass.ts(h, 64)],
                                     e2t[:, u, :],
                                     start=(u == 0), stop=(u == 3))
                posb = est.tile([64, N], F32, tag="posb")
                nc.scalar.copy(posb[:], po[:])
                nc.vector.tensor_tensor(
                    obt[qpp:qpp + 64, qpo, bass.ds(b * N, 512)],
                    posb[:], rdb[:], op=OP.mult)

        # ---------- phase F: output projection ----------
        with tc.tile_pool(name="fps", bufs=4, space="PSUM") as fp, \
             tc.tile_pool(name="o2", bufs=1) as o2p:
            o2 = o2p.tile([128, 4, TOK], F32)     # proj out^T [oc, tok]
            for oc in range(4):
                for tch in range(2):
                    ps = fp.tile([128, 512], F32, tag="pj")
                    for ct in range(4):
                        nc.tensor.matmul(ps[:], wpb[:, ct, bass.ts(oc, 128)],
                                         obt[:, ct, bass.ts(tch, 512)],
                                         start=(ct == 0), stop=(ct == 3))
                    if (oc + tch) % 2:
                        nc.scalar.copy(o2[:, oc, bass.ts(tch, 512)], ps[:])
                    else:
                        nc.vector.tensor_copy(o2[:, oc, bass.ts(tch, 512)],
                                              ps[:])
            nc.sync.dma_start(out_d.rearrange("(k p) t -> p k t", p=128),
                              o2[:])


def _prep_inputs(x, w_qkv, w_proj):
    gum = _gumbel_full()
    eye = _eye_mask()
    wqt = np.ascontiguousarray(w_qkv.T).astype(np.float32).copy()
    wqt[:, :C] *= HD ** -0.5
    wpt = np.ascontiguousarray(w_proj.T).astype(np.float32)
    in_maps = []
    for i in range(NCORES):
        xs = np.ascontiguousarray(
            x[BPC * i:BPC * (i + 1)].reshape(TOK, C).T)
        gs = np.ascontiguousarray(gum[NSL * i:NSL * (i + 1)])
        in_maps.append({
            "xt": xs, "wqt": wqt, "wpt": wpt, "gum": gs, "eye": eye,
        })
    return in_maps


def kernel(x, w_qkv, w_proj, b_proj, _trace=False, _tracedir=None):
    x = np.asarray(x, np.float32)
    w_qkv = np.asarray(w_qkv, np.float32)
    w_proj = np.asarray(w_proj, np.float32)
    b_proj = np.asarray(b_proj, np.float32)
    nc = _build()
    in_maps = _prep_inputs(x, w_qkv, w_proj)
    kw = {}
    if _trace:
        kw = dict(trace=True, tmpdir=_tracedir)
    res = run_bass_kernel_spmd(nc, in_maps, core_ids=list(range(NCORES)), **kw)
    out = np.empty((B, N, C), np.float32)
    for i in range(NCORES):
        ot = np.asarray(res.results[i]["outt"])     # [C, TOK]
        out[BPC * i:BPC * (i + 1)] = ot.T.reshape(BPC, N, C)
    out += b_proj
    if _trace:
        return out, res
    return out


# revision 47
# speedup vs baseline: 2.0501x; 1.0223x over previous
"""Trainium2 Bass kernel for nn_Attention_84215718740239 (sparse attention
with Gumbel top-k mask dropout).

Strategy: data-parallel over batch (2 batches/core x 8 cores = 16 (b,h)
slices per core). All compute on-device except:
  - host-side layout prep (transposes of x / weights for DMA-friendly APs)
  - the Gumbel noise table, which is a data-independent constant
    (jax.random.key(42)) precomputed once on host
  - final un-shard (transpose back + bias add)

Device pipeline per core:
  B: qkv projection -> qT,kT (d-major, f32) and v (natural, bf16)
  C: per slice: attn logits (PE, f32) -> exp/stats/log-prob -> scores
     (scores = log(spatial)+log(aqm)+gumbel, diag pre-killed via the
     gumbel constant)
  D: per-slice top-k THRESHOLD via 6 count-sweeps (2 fixed probes +
     4 secant updates); counts by tensor_scalar(is_ge, accum_out) +
     PE ones-matmul partition reduction
  E: masked softmax applied multiplicatively (E2 = E * (score < thr)),
     DMA-transpose E2 (bf16), attn@v on PE, rows scaled by 1/rowsum
  F: output projection (bf16) producing out^T; host transposes back
"""

import numpy as np

import sys
if '/opt/trn_rl_repo' not in sys.path:
    sys.path.insert(0, '/opt/trn_rl_repo')

import concourse.bass as bass
import concourse.tile as tile
from concourse import bacc, mybir
from concourse.bass_utils import run_bass_kernel_spmd

F32 = mybir.dt.float32
BF16 = mybir.dt.bfloat16
AX = mybir.AxisListType
OP = mybir.AluOpType
AF = mybir.ActivationFunctionType

B, N, C, H, HD = 16, 512, 512, 8, 64
NCORES = 8
BPC = B // NCORES            # batches per core
NSL = BPC * H                # 16 slices per core
LF = N // 2                  # 256
KTARG = float(int(0.1 * (LF - 1)) * N)   # 12800 samples per slice
TOK = BPC * N                # 1024 tokens per core

_gum_cache = None


def _gumbel_full():
    """[128, 512, 256] f32 Gumbel noise (fixed key 42), diag pre-set to -1e30."""
    global _gum_cache
    if _gum_cache is None:
        import jax
        import jax.numpy as jnp
        with jax.default_device(jax.devices('cpu')[0]):
            u = jax.random.uniform(jax.random.key(42), (B * H, N * LF),
                                   dtype=jnp.float32, minval=1e-20, maxval=1.0)
            g = np.asarray(-jnp.log(-jnp.log(u))).reshape(B * H, N, LF).copy()
        g = np.exp(g)                       # exp-domain gumbel factor
        idx = np.arange(LF)
        g[:, idx, idx] = 0.0                # diagonal never sampled
        g[:, LF + idx, idx] = 0.0
        _gum_cache = g
    return _gum_cache


def _eye_mask():
    e = np.zeros((2, 128, LF), np.float32)
    p = np.arange(128)
    e[0, p, p] = 1.0
    e[1, p, 128 + p] = 1.0
    return e


_nc_cache = None
DEBUG_DUMP = False


def _build():
    global _nc_cache
    if _nc_cache is not None:
        return _nc_cache
    nc = bacc.Bacc("TRN2", target_bir_lowering=False, debug=False,
                   num_devices=NCORES)

    xt_d = nc.dram_tensor("xt", [C, TOK], F32, kind="ExternalInput").ap()
    wq_d = nc.dram_tensor("wqt", [C, 3 * C], F32, kind="ExternalInput").ap()
    wp_d = nc.dram_tensor("wpt", [C, C], F32, kind="ExternalInput").ap()
    gum_d = nc.dram_tensor("gum", [NSL, N, LF], F32, kind="ExternalInput").ap()
    eye_d = nc.dram_tensor("eye", [2, 128, LF], F32, kind="ExternalInput").ap()
    sel_d = nc.dram_tensor("sel", [16, 8, 128], F32, kind="ExternalInput").ap()
    out_d = nc.dram_tensor("outt", [C, TOK], F32, kind="ExternalOutput").ap()

    dbg = None
    if DEBUG_DUMP:
        dbg = {}
        for name, shape in DEBUG_DUMP.items():
            dbg[name] = nc.dram_tensor(name, shape, F32,
                                       kind="ExternalOutput").ap()

    with tile.TileContext(nc) as tc:
        _emit(nc, tc, xt_d, wq_d, wp_d, gum_d, eye_d, sel_d, out_d, dbg)
    nc.compile()
    _nc_cache = nc
    return nc


def _emit(nc, tc, xt_d, wq_d, wp_d, gum_d, eye_d, sel_d, out_d, dbg=None):
    from contextlib import ExitStack
    ctx = ExitStack()
    with ctx:
        const = ctx.enter_context(tc.tile_pool(name="const", bufs=1))
        stats = ctx.enter_context(tc.tile_pool(name="stats", bufs=1))

        # ---------- load inputs ----------
        eye = const.tile([128, 2, LF], F32)
        nc.sync.dma_start(eye[:], eye_d.rearrange("e p c -> p e c"))
        selc = const.tile([16, 8, 128], F32)
        nc.sync.dma_start(selc[:], sel_d)

        ones_col = const.tile([128, 1], F32)      # lhsT for partition-sum
        nc.vector.memset(ones_col[:], 1.0)
        ones_row = const.tile([1, 128], F32)      # lhsT for partition-bcast
        nc.vector.memset(ones_row[:], 1.0)
        one16 = const.tile([1, NSL], F32)
        nc.vector.memset(one16[:], 1.0)
        ones64 = const.tile([1, 64], F32)
        nc.vector.memset(ones64[:], 1.0)
        ones_col_bf = const.tile([128, 1], BF16)
        nc.vector.memset(ones_col_bf[:], 1.0)
        idbf = const.tile([128, 128], BF16)
        from concourse import masks as _masks
        _masks.make_identity(nc, idbf[:])
        c1em6 = const.tile([128, 1], F32)
        nc.vector.memset(c1em6[:], 1e-6)
        # count-conversion vectors: slices 0-7 counted on DVE (n = cnt),
        # slices 8-15 on ACT via Sign (n = 0.5*sgnsum + 65536)
        a16 = const.tile([1, NSL], F32)
        nc.vector.memset(a16[:, :8], 1.0)
        nc.vector.memset(a16[:, 8:], 0.5)
        b16 = const.tile([1, NSL], F32)
        nc.vector.memset(b16[:, :8], 0.0)
        nc.vector.memset(b16[:, 8:], float(N * LF / 2))

        wpb = const.tile([128, 4, C], BF16)
        vbf = const.tile([128, 8, C], BF16)       # v natural [tok, vc]
        so_all = stats.tile([128, NSL, 4], F32)   # opp-half exp row-sums
        thr = stats.tile([128, NSL], F32)         # bcast thresholds
        rd2_all = stats.tile([NSL, N], F32)       # 1/rowsum after masking
        cnt0 = stats.tile([128, NSL], F32)        # probe counts at exp(-9.1)
        cnt1 = stats.tile([128, NSL], F32)        # probe counts at exp(-8.8)
        import math
        thr0c = const.tile([128, 1], F32)
        nc.vector.memset(thr0c[:], math.exp(-9.10))
        thr1c = const.tile([128, 1], F32)
        nc.vector.memset(thr1c[:], math.exp(-8.80))
        nthr0c = const.tile([128, 1], F32)
        nc.vector.memset(nthr0c[:], -math.exp(-9.10))
        nthr1c = const.tile([128, 1], F32)
        nc.vector.memset(nthr1c[:], -math.exp(-8.80))

        F32R = mybir.dt.float32r
        qkpool = ctx.enter_context(tc.tile_pool(name="qkp", bufs=1))
        qk = qkpool.tile([128, 8, TOK], F32R)     # q^T,k^T [outc, tok]

        # ---------- phase B: qkv projection ----------
        with tc.tile_pool(name="pbps", bufs=4, space="PSUM") as pb, \
             tc.tile_pool(name="pin", bufs=1) as pin:
            xt0 = pin.tile([128, 4, TOK], F32)    # x^T  [c, tok]
            nc.sync.dma_start(xt0[:],
                              xt_d.rearrange("(k p) t -> p k t", p=128))
            wq0 = pin.tile([128, 4, 3 * C], F32)  # w_qkv^T [c, outc]
            nc.sync.dma_start(wq0[:],
                              wq_d.rearrange("(k p) t -> p k t", p=128))
            wp = pin.tile([128, 4, C], F32)       # w_proj^T [c, oc]
            nc.sync.dma_start(wp[:], wp_d.rearrange("(k p) t -> p k t", p=128))
            for k in range(4):
                nc.vector.tensor_copy(wpb[:, k, :], wp[:, k, :])
            xt = pin.tile([128, 4, TOK], F32R)    # rounded for PE f32r
            wq = pin.tile([128, 4, 3 * C], F32R)
            for k in range(4):
                nc.vector.tensor_copy(xt[:, k, :], xt0[:, k, :])
                nc.scalar.copy(wq[:, k, :], wq0[:, k, :])
            for po in range(8):                   # outc tiles of q,k
                for tch in range(2):
                    ps = pb.tile([128, 512], F32)
                    for kc in range(4):
                        nc.tensor.matmul(
                            ps[:],
                            wq[:, kc, bass.ts(po, 128)],
                            xt[:, kc, bass.ts(tch, 512)],
                            start=(kc == 0), stop=(kc == 3))
                    if (po + tch) % 2:
                        nc.scalar.copy(qk[:, po, bass.ts(tch, 512)], ps[:])
                    else:
                        nc.vector.tensor_copy(qk[:, po, bass.ts(tch, 512)],
                                              ps[:])
            for tt in range(8):                   # v tok tiles
                ps = pb.tile([128, 512], F32)
                for kc in range(4):
                    nc.tensor.matmul(
                        ps[:],
                        xt[:, kc, bass.ts(tt, 128)],
                        wq[:, kc, bass.ds(2 * C, 512)],
                        start=(kc == 0), stop=(kc == 3))
                if tt % 2:
                    nc.scalar.copy(vbf[:, tt, :], ps[:])
                else:
                    nc.vector.tensor_copy(vbf[:, tt, :], ps[:])

        # persistent big tensors (allocated after phase-B scratch is freed)
        big = ctx.enter_context(tc.tile_pool(name="big", bufs=1))
        sc = big.tile([128, NSL, 4 * LF], F32)    # scores per slice

        # ---------- phase C: logits -> scores per slice ----------
        with tc.tile_pool(name="cps", bufs=4, space="PSUM") as cp, \
             tc.tile_pool(name="ef", bufs=2) as efp, \
             tc.tile_pool(name="gp", bufs=1) as gp, \
             tc.tile_pool(name="st4", bufs=4) as st4, \
             tc.tile_pool(name="gj", bufs=2) as gjp:
            for s in range(NSL):
                b, h = divmod(s, H)
                qpo, qpp = h // 2, 64 * (h % 2)
                gt = gp.tile([128, 4, LF], F32, tag="g")
                nc.sync.dma_start(
                    gt[:], gum_d[s].rearrange("(t p) c -> p t c", p=128))
                ssr4 = st4.tile([128, 4], F32, tag="ssr")
                so4v = so_all[:, s, :]
                eii4 = st4.tile([128, 4], F32, tag="eii")
                mx4 = st4.tile([128, 4], F32, tag="mx")
                ef = efp.tile([128, 4, 512], F32, tag="ef")
                for t in range(4):
                    soff = 0 if t < 2 else LF
                    ooff = LF - soff
                    ps = cp.tile([128, 512], F32, tag="attn")
                    lhs = qk[qpp:qpp + 64, qpo, bass.ds(b * N + t * 128, 128)]
                    rhs = qk[qpp:qpp + 64, 4 + qpo, bass.ds(b * N, 512)]
                    nc.tensor.matmul(ps[:], lhs, rhs, start=True, stop=True)
                    # row max of opposite block (logit domain, f32)
                    nc.vector.tensor_reduce(mx4[:, t:t + 1],
                                            ps[:, ooff:ooff + LF],
                                            axis=AX.X, op=OP.max)
                    nc.scalar.activation(ef[:, t, soff:soff + LF],
                                         ps[:, soff:soff + LF], AF.Exp,
                                         accum_out=ssr4[:, t:t + 1])
                    nc.scalar.activation(ef[:, t, ooff:ooff + LF],
                                         ps[:, ooff:ooff + LF], AF.Exp,
                                         accum_out=so4v[:, t:t + 1])
                    # diagonal element of the same-block
                    gj = gjp.tile([128, LF], F32, tag="gj")
                    nc.vector.scalar_tensor_tensor(
                        gj[:], ef[:, t, soff:soff + LF], 1.0, eye[:, t % 2, :],
                        op0=OP.mult, op1=OP.mult,
                        accum_out=eii4[:, t:t + 1])
                # per-row scalars for all 4 tiles at once  [128,4]
                d4 = st4.tile([128, 4], F32, tag="d4")
                nc.vector.tensor_tensor(d4[:], ssr4[:], so4v, op=OP.add)
                rd4 = st4.tile([128, 4], F32, tag="rd4")
                nc.vector.reciprocal(rd4[:], d4[:])
                sv4 = st4.tile([128, 4], F32, tag="sv4")
                nc.vector.tensor_tensor(sv4[:], ssr4[:], eii4[:],
                                        op=OP.subtract)
                nc.vector.tensor_tensor(sv4[:], sv4[:], rd4[:], op=OP.mult)
                nc.vector.tensor_scalar(sv4[:], sv4[:], float((LF - 1) * 1e-6),
                                        None, op0=OP.add)
                aqm4 = st4.tile([128, 4], F32, tag="aqm")
                nc.scalar.activation(aqm4[:], mx4[:], AF.Exp)
                nc.vector.tensor_tensor(aqm4[:], aqm4[:], rd4[:], op=OP.mult)
                nc.vector.tensor_scalar(aqm4[:], aqm4[:], 1e-6, None,
                                        op0=OP.add)
                # rq = aqm / S  (exp-domain row factor; no logs anywhere)
                rq4 = st4.tile([128, 4], F32, tag="rq4")
                nc.vector.reciprocal(rq4[:], sv4[:])
                nc.vector.tensor_tensor(rq4[:], rq4[:], aqm4[:], op=OP.mult)
                for t in range(4):
                    soff = 0 if t < 2 else LF
                    slot = sc[:, s, bass.ts(t, LF)]
                    # aw' = E*recipD + 1e-6 ; zsc = aw' * rq * exp(g)
                    aw1 = gjp.tile([128, LF], F32, tag="gj")
                    nc.scalar.activation(aw1[:], ef[:, t, soff:soff + LF],
                                         AF.Identity, scale=rd4[:, t:t + 1],
                                         bias=c1em6[:])
                    nc.vector.scalar_tensor_tensor(
                        slot, aw1[:], rq4[:, t:t + 1], gt[:, t, :],
                        op0=OP.mult, op1=OP.mult)
                # overlapped fixed-probe count sweeps for the selection
                if s < 8:
                    jc = gp.tile([128, 4 * LF], BF16, tag="jc")
                    nc.vector.tensor_scalar(jc[:], sc[:, s, :], thr0c[:], 0.0,
                                            op0=OP.is_ge, op1=OP.add,
                                            accum_out=cnt0[:, s:s + 1])
                    jc = gp.tile([128, 4 * LF], BF16, tag="jc")
                    nc.vector.tensor_scalar(jc[:], sc[:, s, :], thr1c[:], 0.0,
                                            op0=OP.is_ge, op1=OP.add,
                                            accum_out=cnt1[:, s:s + 1])
                else:
                    jc = gp.tile([128, 4 * LF], BF16, tag="jc")
                    nc.scalar.activation(jc[:], sc[:, s, :], AF.Sign,
                                         bias=nthr0c[:],
                                         accum_out=cnt0[:, s:s + 1])
                    jc = gp.tile([128, 4 * LF], BF16, tag="jc")
                    nc.scalar.activation(jc[:], sc[:, s, :], AF.Sign,
                                         bias=nthr1c[:],
                                         accum_out=cnt1[:, s:s + 1])

        # ---------- phase D: threshold selection (secant, 6 sweeps) ----------
        with tc.tile_pool(name="dps", bufs=2, space="PSUM") as dps, \
             tc.tile_pool(name="dbc", bufs=2, space="PSUM") as dbc, \
             tc.tile_pool(name="sel", bufs=1) as sel, \
             tc.tile_pool(name="junk", bufs=2) as jp, \
             tc.tile_pool(name="s16", bufs=8) as s16:
            cnt128 = sel.tile([128, NSL], F32)
            thrN = sel.tile([128, NSL], F32)
            t0 = sel.tile([1, NSL], F32)
            t1 = sel.tile([1, NSL], F32)
            tneg = sel.tile([1, NSL], F32)
            n0 = sel.tile([1, NSL], F32)
            n1 = sel.tile([1, NSL], F32)
            nc.vector.memset(t0[:], math.exp(-9.10))
            nc.vector.memset(t1[:], math.exp(-8.80))

            def count_into(ndst):
                for s in range(8):
                    jk = jp.tile([128, 4 * LF], BF16, tag="junk")
                    nc.vector.tensor_scalar(jk[:], sc[:, s, :],
                                            thr[:, s:s + 1], 0.0,
                                            op0=OP.is_ge, op1=OP.add,
                                            accum_out=cnt128[:, s:s + 1])
                for s in range(8, NSL):
                    jk2 = jp.tile([128, 4 * LF], BF16, tag="junk2")
                    nc.scalar.activation(jk2[:], sc[:, s, :], AF.Sign,
                                         bias=thrN[:, s:s + 1],
                                         accum_out=cnt128[:, s:s + 1])
                cp_ = dps.tile([1, NSL], F32, tag="cnt")
                nc.tensor.matmul(cp_[:], ones_col[:], cnt128[:],
                                 start=True, stop=True)
                nc.vector.tensor_tensor(ndst, cp_[:], a16[:], op=OP.mult)
                nc.vector.tensor_tensor(ndst, ndst, b16[:], op=OP.add)

            def bcast(tsrc):
                bp = dbc.tile([128, NSL], F32, tag="bc")
                nc.tensor.matmul(bp[:], ones_row[:], tsrc, start=True,
                                 stop=True)
                nc.vector.tensor_copy(thr[:], bp[:])
                nc.vector.tensor_scalar(tneg[:], tsrc, -1.0, None,
                                        op0=OP.mult)
                bp2 = dbc.tile([128, NSL], F32, tag="bc2")
                nc.tensor.matmul(bp2[:], ones_row[:], tneg[:], start=True,
                                 stop=True)
                nc.vector.tensor_copy(thrN[:], bp2[:])

            for csrc, ndst in ((cnt0, n0), (cnt1, n1)):
                cp_ = dps.tile([1, NSL], F32, tag="cnt")
                nc.tensor.matmul(cp_[:], ones_col[:], csrc[:],
                                 start=True, stop=True)
                nc.vector.tensor_tensor(ndst[:], cp_[:], a16[:], op=OP.mult)
                nc.vector.tensor_tensor(ndst[:], ndst[:], b16[:], op=OP.add)
            for r in range(4):
                d = s16.tile([1, NSL], F32, tag="d")
                nc.vector.tensor_tensor(d[:], n1[:], n0[:], op=OP.subtract)
                dsq = s16.tile([1, NSL], F32, tag="dsq")
                nc.vector.tensor_tensor(dsq[:], d[:], d[:], op=OP.mult)
                msk = s16.tile([1, NSL], F32, tag="msk")
                nc.vector.tensor_scalar(msk[:], dsq[:], 1.0, None,
                                        op0=OP.is_lt)
                nc.vector.tensor_tensor(d[:], d[:], msk[:], op=OP.subtract)
                rcd = s16.tile([1, NSL], F32, tag="rcd")
                nc.vector.reciprocal(rcd[:], d[:])
                dt = s16.tile([1, NSL], F32, tag="dt")
                nc.vector.tensor_tensor(dt[:], t1[:], t0[:], op=OP.subtract)
                nk = s16.tile([1, NSL], F32, tag="nk")
                nc.vector.tensor_scalar(nk[:], n1[:], KTARG, None,
                                        op0=OP.subtract)
                nc.vector.tensor_tensor(nk[:], nk[:], dt[:], op=OP.mult)
                nc.vector.tensor_tensor(nk[:], nk[:], rcd[:], op=OP.mult)
                t2 = s16.tile([1, NSL], F32, tag="t2")
                nc.vector.tensor_tensor(t2[:], t1[:], nk[:], op=OP.subtract)
                nc.vector.tensor_scalar(t2[:], t2[:], math.exp(-8.0),
                                        math.exp(-10.0),
                                        op0=OP.min, op1=OP.max)
                nc.vector.tensor_copy(t0[:], t1[:])
                nc.vector.tensor_copy(n0[:], n1[:])
                nc.vector.tensor_copy(t1[:], t2[:])
                bcast(t1[:])
                count_into(n1[:])
            if dbg is not None and "dbg_n" in dbg:
                nn = sel.tile([1, 2 * NSL], F32)
                nc.vector.tensor_copy(nn[:, :NSL], n0[:])
                nc.vector.tensor_copy(nn[:, NSL:], n1[:])
                nc.sync.dma_start(dbg["dbg_n"], nn[:])

        if dbg is not None:
            srcs = {"dbg_sc": sc, "dbg_thr": thr, "dbg_so": so_all,
                    "dbg_qk": qk}
            for name, t_ in srcs.items():
                if name in dbg:
                    nc.sync.dma_start(dbg[name], t_[:].bitcast(F32))

        # ---------- phase E: masked softmax + attn@v (transposed) ----------
        obtp = ctx.enter_context(tc.tile_pool(name="obt", bufs=1))
        obt = obtp.tile([128, 4, TOK], BF16)      # attn-out^T [c, tok]
        with tc.tile_pool(name="eaT", bufs=2, space="PSUM") as eaT, \
             tc.tile_pool(name="ekt", bufs=2, space="PSUM") as ektp, \
             tc.tile_pool(name="ed2", bufs=1, space="PSUM") as ed2p, \
             tc.tile_pool(name="epo", bufs=2, space="PSUM") as epop, \
             tc.tile_pool(name="e2tp", bufs=2) as e2tp, \
             tc.tile_pool(name="ek01", bufs=2) as k01p, \
             tc.tile_pool(name="est", bufs=2) as est:
            for s in range(NSL):
                b, h = divmod(s, H)
                qpo, qpp = h // 2, 64 * (h % 2)
                e2t = e2tp.tile([128, 4, N], BF16, tag="e2t")
                # attn^T per kcol-tile u  ->  E^T = exp (bf16)
                for u in range(4):
                    psT = eaT.tile([128, 512], F32, tag="aT")
                    lhs = qk[qpp:qpp + 64, 4 + qpo,
                             bass.ds(b * N + u * 128, 128)]
                    rhs = qk[qpp:qpp + 64, qpo, bass.ds(b * N, 512)]
                    nc.tensor.matmul(psT[:], lhs, rhs, start=True, stop=True)
                    nc.scalar.activation(e2t[:, u, :], psT[:], AF.Exp)
                # keep mask (natural), 0/1 bf16
                k01 = k01p.tile([128, 4, LF], BF16, tag="k01")
                for t in range(4):
                    nc.vector.tensor_scalar(k01[:, t, :],
                                            sc[:, s, bass.ts(t, LF)],
                                            thr[:, s:s + 1], None,
                                            op0=OP.is_lt)
                # transpose keep blocks on PE, apply to diag quadrants
                for t in range(4):
                    qoff = 0 if t < 2 else LF     # qrow quadrant offset
                    for uu in range(2):
                        u = 2 * (t // 2) + uu
                        ktp = ektp.tile([128, 128], BF16, tag="ktp")
                        nc.tensor.transpose(ktp[:],
                                            k01[:, t, bass.ts(uu, 128)],
                                            idbf[:])
                        dst = e2t[:, u, qoff + (t % 2) * 128:
                                  qoff + (t % 2) * 128 + 128]
                        nc.vector.tensor_tensor(dst, dst, ktp[:], op=OP.mult)
                # D2 row-sums via ones-matmul over kcol partitions
                d2 = ed2p.tile([1, N], F32, tag="d2")
                for u in range(4):
                    nc.tensor.matmul(d2[:], ones_col_bf[:], e2t[:, u, :],
                                     start=(u == 0), stop=(u == 3))
                d2sb = est.tile([1, N], F32, tag="d2sb")
                nc.scalar.copy(d2sb[:], d2[:])
                nc.gpsimd.dma_start(rd2_all[s:s + 1, :], d2sb[:])
                # out^T = v^T @ p^T  (accumulate over kcol chunks)
                po = epop.tile([64, N], F32, tag="po")
                for u in range(4):
                    nc.tensor.matmul(po[:],
                                     vbf[:, 4 * b + u, bass.ts(h, 64)],
                                     e2t[:, u, :],
                                     start=(u == 0), stop=(u == 3))
                nc.scalar.copy(obt[qpp:qpp + 64, qpo, bass.ds(b * N, 512)],
                               po[:])
            # batched reciprocal of all row-sums (16 partitions in parallel)
            nc.vector.reciprocal(rd2_all[:], rd2_all[:])

        # ---------- phase F: output projection ----------
        with tc.tile_pool(name="fps", bufs=4, space="PSUM") as fp, \
             tc.tile_pool(name="o2", bufs=1) as o2p:
            for ct in range(4):
                for bb in range(2):
                    rdbf = fp.tile([128, 512], F32, tag="rdbf")
                    nc.tensor.matmul(rdbf[:], selc[:, ct * 2 + bb, :],
                                     rd2_all[:], start=True, stop=True)
                    dst = obt[:, ct, bass.ts(bb, 512)]
                    nc.vector.tensor_tensor(dst, dst, rdbf[:], op=OP.mult)
            o2 = o2p.tile([128, 4, TOK], F32)     # proj out^T [oc, tok]
            for oc in range(4):
                for tch in range(2):
                    ps = fp.tile([128, 512], F32, tag="pj")
                    for ct in range(4):
                        nc.tensor.matmul(ps[:], wpb[:, ct, bass.ts(oc, 128)],
                                         obt[:, ct, bass.ts(tch, 512)],
                                         start=(ct == 0), stop=(ct == 3))
                    if (oc + tch) % 2:
                        nc.scalar.copy(o2[:, oc, bass.ts(tch, 512)], ps[:])
                    else:
                        nc.vector.tensor_copy(o2[:, oc, bass.ts(tch, 512)],
                                              ps[:])
            nc.sync.dma_start(out_d.rearrange("(k p) t -> p k t", p=128),
                              o2[:])


def _sel_mask():
    sel = np.zeros((16, 8, 128), np.float32)
    for ct in range(4):
        for bb in range(2):
            for p in range(128):
                sel[bb * 8 + 2 * ct + p // 64, ct * 2 + bb, p] = 1.0
    return sel


def _prep_inputs(x, w_qkv, w_proj):
    gum = _gumbel_full()
    eye = _eye_mask()
    sel = _sel_mask()
    wqt = np.ascontiguousarray(w_qkv.T).astype(np.float32).copy()
    wqt[:, :C] *= HD ** -0.5
    wpt = np.ascontiguousarray(w_proj.T).astype(np.float32)
    in_maps = []
    for i in range(NCORES):
        xs = np.ascontiguousarray(
            x[BPC * i:BPC * (i + 1)].reshape(TOK, C).T)
        gs = np.ascontiguousarray(gum[NSL * i:NSL * (i + 1)])
        in_maps.append({
            "xt": xs, "wqt": wqt, "wpt": wpt, "gum": gs, "eye": eye,
            "sel": sel,
        })
    return in_maps


def kernel(x, w_qkv, w_proj, b_proj, _trace=False, _tracedir=None):
    x = np.asarray(x, np.float32)
    w_qkv = np.asarray(w_qkv, np.float32)
    w_proj = np.asarray(w_proj, np.float32)
    b_proj = np.asarray(b_proj, np.float32)
    nc = _build()
    in_maps = _prep_inputs(x, w_qkv, w_proj)
    kw = {}
    if _trace:
        kw = dict(trace=True, tmpdir=_tracedir)
    res = run_bass_kernel_spmd(nc, in_maps, core_ids=list(range(NCORES)), **kw)
    out = np.empty((B, N, C), np.float32)
    for i in range(NCORES):
        ot = np.asarray(res.results[i]["outt"])     # [C, TOK]
        out[BPC * i:BPC * (i + 1)] = ot.T.reshape(BPC, N, C)
    out += b_proj
    if _trace:
        return out, res
    return out


# revision 49
# speedup vs baseline: 2.4626x; 1.2012x over previous
"""Trainium2 Bass kernel for nn_Attention_84215718740239 (sparse attention
with Gumbel top-k mask dropout).

Strategy: data-parallel over batch (2 batches/core x 8 cores = 16 (b,h)
slices per core). All compute on-device except:
  - host-side layout prep (transposes of x / weights for DMA-friendly APs)
  - the Gumbel noise table, which is a data-independent constant
    (jax.random.key(42)) precomputed once on host
  - final un-shard (transpose back + bias add)

Device pipeline per core:
  B: qkv projection -> qT,kT (d-major, f32) and v (natural, bf16)
  C: per slice: attn logits (PE, f32) -> exp/stats/log-prob -> scores
     (scores = log(spatial)+log(aqm)+gumbel, diag pre-killed via the
     gumbel constant)
  D: per-slice top-k THRESHOLD via 6 count-sweeps (2 fixed probes +
     4 secant updates); counts by tensor_scalar(is_ge, accum_out) +
     PE ones-matmul partition reduction
  E: masked softmax applied multiplicatively (E2 = E * (score < thr)),
     DMA-transpose E2 (bf16), attn@v on PE, rows scaled by 1/rowsum
  F: output projection (bf16) producing out^T; host transposes back
"""

import numpy as np

import sys
if '/opt/trn_rl_repo' not in sys.path:
    sys.path.insert(0, '/opt/trn_rl_repo')

import concourse.bass as bass
import concourse.tile as tile
from concourse import bacc, mybir
from concourse.bass_utils import run_bass_kernel_spmd

F32 = mybir.dt.float32
BF16 = mybir.dt.bfloat16
AX = mybir.AxisListType
OP = mybir.AluOpType
AF = mybir.ActivationFunctionType

B, N, C, H, HD = 16, 512, 512, 8, 64
NCORES = 8
BPC = B // NCORES            # batches per core
NSL = BPC * H                # 16 slices per core
LF = N // 2                  # 256
KTARG = float(int(0.1 * (LF - 1)) * N)   # 12800 samples per slice
TOK = BPC * N                # 1024 tokens per core

_gum_cache = None


def _gumbel_full():
    """[128, 512, 256] f32 Gumbel noise (fixed key 42), diag pre-set to -1e30."""
    global _gum_cache
    if _gum_cache is None:
        import jax
        import jax.numpy as jnp
        with jax.default_device(jax.devices('cpu')[0]):
            u = jax.random.uniform(jax.random.key(42), (B * H, N * LF),
                                   dtype=jnp.float32, minval=1e-20, maxval=1.0)
            g = np.asarray(-jnp.log(-jnp.log(u))).reshape(B * H, N, LF).copy()
        g = np.exp(g)                       # exp-domain gumbel factor
        idx = np.arange(LF)
        g[:, idx, idx] = 0.0                # diagonal never sampled
        g[:, LF + idx, idx] = 0.0
        _gum_cache = g
    return _gum_cache


def _eye_mask():
    e = np.zeros((2, 128, LF), np.float32)
    p = np.arange(128)
    e[0, p, p] = 1.0
    e[1, p, 128 + p] = 1.0
    return e


_nc_cache = None
DEBUG_DUMP = False


def _build():
    global _nc_cache
    if _nc_cache is not None:
        return _nc_cache
    nc = bacc.Bacc("TRN2", target_bir_lowering=False, debug=False,
                   num_devices=NCORES)

    xt_d = nc.dram_tensor("xt", [C, TOK], F32, kind="ExternalInput").ap()
    wq_d = nc.dram_tensor("wqt", [C, 3 * C], F32, kind="ExternalInput").ap()
    wp_d = nc.dram_tensor("wpt", [C, C], F32, kind="ExternalInput").ap()
    gum_d = nc.dram_tensor("gum", [NSL, N, LF], F32, kind="ExternalInput").ap()
    eye_d = nc.dram_tensor("eye", [2, 128, LF], F32, kind="ExternalInput").ap()
    sel_d = nc.dram_tensor("sel", [16, 8, 128], F32, kind="ExternalInput").ap()
    out_d = nc.dram_tensor("outt", [C, TOK], F32, kind="ExternalOutput").ap()

    dbg = None
    if DEBUG_DUMP:
        dbg = {}
        for name, shape in DEBUG_DUMP.items():
            dbg[name] = nc.dram_tensor(name, shape, F32,
                                       kind="ExternalOutput").ap()

    with tile.TileContext(nc) as tc:
        _emit(nc, tc, xt_d, wq_d, wp_d, gum_d, eye_d, sel_d, out_d, dbg)
    nc.compile()
    _nc_cache = nc
    return nc


def _emit(nc, tc, xt_d, wq_d, wp_d, gum_d, eye_d, sel_d, out_d, dbg=None):
    from contextlib import ExitStack
    ctx = ExitStack()
    with ctx:
        const = ctx.enter_context(tc.tile_pool(name="const", bufs=1))
        stats = ctx.enter_context(tc.tile_pool(name="stats", bufs=1))

        # ---------- load inputs ----------
        eye = const.tile([128, 2, LF], F32)
        nc.sync.dma_start(eye[:], eye_d.rearrange("e p c -> p e c"))
        selc = const.tile([16, 8, 128], F32)
        nc.sync.dma_start(selc[:], sel_d)

        ones_col = const.tile([128, 1], F32)      # lhsT for partition-sum
        nc.vector.memset(ones_col[:], 1.0)
        ones_row = const.tile([1, 128], F32)      # lhsT for partition-bcast
        nc.vector.memset(ones_row[:], 1.0)
        one16 = const.tile([1, NSL], F32)
        nc.vector.memset(one16[:], 1.0)
        ones64 = const.tile([1, 64], F32)
        nc.vector.memset(ones64[:], 1.0)
        ones_col_bf = const.tile([128, 1], BF16)
        nc.vector.memset(ones_col_bf[:], 1.0)
        idbf = const.tile([128, 128], BF16)
        from concourse import masks as _masks
        _masks.make_identity(nc, idbf[:])
        c1em6 = const.tile([128, 1], F32)
        nc.vector.memset(c1em6[:], 1e-6)
        # count-conversion vectors: slices 0-7 counted on DVE (n = cnt),
        # slices 8-15 on ACT via Sign (n = 0.5*sgnsum + 65536)
        a16 = const.tile([1, NSL], F32)
        nc.vector.memset(a16[:, :8], 1.0)
        nc.vector.memset(a16[:, 8:], 0.5)
        b16 = const.tile([1, NSL], F32)
        nc.vector.memset(b16[:, :8], 0.0)
        nc.vector.memset(b16[:, 8:], float(N * LF / 2))

        wpb = const.tile([128, 4, C], BF16)
        vbf = const.tile([128, 8, C], BF16)       # v natural [tok, vc]
        so_all = stats.tile([128, NSL, 4], F32)   # opp-half exp row-sums
        thr = stats.tile([128, NSL], F32)         # bcast thresholds
        rd2_all = stats.tile([NSL, N], F32)       # 1/rowsum after masking
        cnt0 = stats.tile([128, NSL], F32)        # probe counts at exp(-9.1)
        cnt1 = stats.tile([128, NSL], F32)        # probe counts at exp(-8.8)
        import math
        thr0c = const.tile([128, 1], F32)
        nc.vector.memset(thr0c[:], math.exp(-9.10))
        thr1c = const.tile([128, 1], F32)
        nc.vector.memset(thr1c[:], math.exp(-8.80))
        nthr0c = const.tile([128, 1], F32)
        nc.vector.memset(nthr0c[:], -math.exp(-9.10))
        nthr1c = const.tile([128, 1], F32)
        nc.vector.memset(nthr1c[:], -math.exp(-8.80))

        F32R = mybir.dt.float32r
        qkpool = ctx.enter_context(tc.tile_pool(name="qkp", bufs=1))
        qk = qkpool.tile([128, 8, TOK], F32R)     # q^T,k^T [outc, tok]

        # ---------- phase B: qkv projection ----------
        with tc.tile_pool(name="pbps", bufs=4, space="PSUM") as pb, \
             tc.tile_pool(name="pin", bufs=1) as pin:
            xt0 = pin.tile([128, 4, TOK], F32)    # x^T  [c, tok]
            nc.sync.dma_start(xt0[:],
                              xt_d.rearrange("(k p) t -> p k t", p=128))
            wq0 = pin.tile([128, 4, 3 * C], F32)  # w_qkv^T [c, outc]
            nc.sync.dma_start(wq0[:],
                              wq_d.rearrange("(k p) t -> p k t", p=128))
            wp = pin.tile([128, 4, C], F32)       # w_proj^T [c, oc]
            nc.sync.dma_start(wp[:], wp_d.rearrange("(k p) t -> p k t", p=128))
            for k in range(4):
                nc.vector.tensor_copy(wpb[:, k, :], wp[:, k, :])
            xt = pin.tile([128, 4, TOK], F32R)    # rounded for PE f32r
            wq = pin.tile([128, 4, 3 * C], F32R)
            for k in range(4):
                nc.vector.tensor_copy(xt[:, k, :], xt0[:, k, :])
                nc.scalar.copy(wq[:, k, :], wq0[:, k, :])
            for po in range(8):                   # outc tiles of q,k
                for tch in range(2):
                    ps = pb.tile([128, 512], F32)
                    for kc in range(4):
                        nc.tensor.matmul(
                            ps[:],
                            wq[:, kc, bass.ts(po, 128)],
                            xt[:, kc, bass.ts(tch, 512)],
                            start=(kc == 0), stop=(kc == 3))
                    if (po + tch) % 2:
                        nc.scalar.copy(qk[:, po, bass.ts(tch, 512)], ps[:])
                    else:
                        nc.vector.tensor_copy(qk[:, po, bass.ts(tch, 512)],
                                              ps[:])
            for tt in range(8):                   # v tok tiles
                ps = pb.tile([128, 512], F32)
                for kc in range(4):
                    nc.tensor.matmul(
                        ps[:],
                        xt[:, kc, bass.ts(tt, 128)],
                        wq[:, kc, bass.ds(2 * C, 512)],
                        start=(kc == 0), stop=(kc == 3))
                if tt % 2:
                    nc.scalar.copy(vbf[:, tt, :], ps[:])
                else:
                    nc.vector.tensor_copy(vbf[:, tt, :], ps[:])

        # persistent big tensors (allocated after phase-B scratch is freed)
        big = ctx.enter_context(tc.tile_pool(name="big", bufs=1))
        sc = big.tile([128, NSL, 4 * LF], F32)    # scores per slice
        e2t_all = big.tile([128, NSL, 4, N], BF16)  # exp(attn^T) per slice

        # ---------- phase C: logits -> scores per slice ----------
        with tc.tile_pool(name="cps", bufs=4, space="PSUM") as cp, \
             tc.tile_pool(name="caT", bufs=2, space="PSUM") as eaT, \
             tc.tile_pool(name="ef", bufs=2) as efp, \
             tc.tile_pool(name="gp", bufs=1) as gp, \
             tc.tile_pool(name="st4", bufs=4) as st4, \
             tc.tile_pool(name="gj", bufs=2) as gjp:
            for s in range(NSL):
                b, h = divmod(s, H)
                qpo, qpp = h // 2, 64 * (h % 2)
                gt = gp.tile([128, 4, LF], F32, tag="g")
                nc.sync.dma_start(
                    gt[:], gum_d[s].rearrange("(t p) c -> p t c", p=128))
                ssr4 = st4.tile([128, 4], F32, tag="ssr")
                so4v = so_all[:, s, :]
                eii4 = st4.tile([128, 4], F32, tag="eii")
                mx4 = st4.tile([128, 4], F32, tag="mx")
                ef = efp.tile([128, 4, 512], F32, tag="ef")
                for t in range(4):
                    soff = 0 if t < 2 else LF
                    ooff = LF - soff
                    ps = cp.tile([128, 512], F32, tag="attn")
                    lhs = qk[qpp:qpp + 64, qpo, bass.ds(b * N + t * 128, 128)]
                    rhs = qk[qpp:qpp + 64, 4 + qpo, bass.ds(b * N, 512)]
                    nc.tensor.matmul(ps[:], lhs, rhs, start=True, stop=True)
                    # row max of opposite block (logit domain, f32)
                    nc.vector.tensor_reduce(mx4[:, t:t + 1],
                                            ps[:, ooff:ooff + LF],
                                            axis=AX.X, op=OP.max)
                    nc.scalar.activation(ef[:, t, soff:soff + LF],
                                         ps[:, soff:soff + LF], AF.Exp,
                                         accum_out=ssr4[:, t:t + 1])
                    nc.scalar.activation(ef[:, t, ooff:ooff + LF],
                                         ps[:, ooff:ooff + LF], AF.Exp,
                                         accum_out=so4v[:, t:t + 1])
                    # diagonal element of the same-block
                    gj = gjp.tile([128, LF], F32, tag="gj")
                    nc.vector.scalar_tensor_tensor(
                        gj[:], ef[:, t, soff:soff + LF], 1.0, eye[:, t % 2, :],
                        op0=OP.mult, op1=OP.mult,
                        accum_out=eii4[:, t:t + 1])
                # per-row scalars for all 4 tiles at once  [128,4]
                d4 = st4.tile([128, 4], F32, tag="d4")
                nc.vector.tensor_tensor(d4[:], ssr4[:], so4v, op=OP.add)
                rd4 = st4.tile([128, 4], F32, tag="rd4")
                nc.vector.reciprocal(rd4[:], d4[:])
                sv4 = st4.tile([128, 4], F32, tag="sv4")
                nc.vector.tensor_tensor(sv4[:], ssr4[:], eii4[:],
                                        op=OP.subtract)
                nc.vector.tensor_tensor(sv4[:], sv4[:], rd4[:], op=OP.mult)
                nc.vector.tensor_scalar(sv4[:], sv4[:], float((LF - 1) * 1e-6),
                                        None, op0=OP.add)
                aqm4 = st4.tile([128, 4], F32, tag="aqm")
                nc.scalar.activation(aqm4[:], mx4[:], AF.Exp)
                nc.vector.tensor_tensor(aqm4[:], aqm4[:], rd4[:], op=OP.mult)
                nc.vector.tensor_scalar(aqm4[:], aqm4[:], 1e-6, None,
                                        op0=OP.add)
                # rq = aqm / S  (exp-domain row factor; no logs anywhere)
                rq4 = st4.tile([128, 4], F32, tag="rq4")
                nc.vector.reciprocal(rq4[:], sv4[:])
                nc.vector.tensor_tensor(rq4[:], rq4[:], aqm4[:], op=OP.mult)
                for t in range(4):
                    soff = 0 if t < 2 else LF
                    slot = sc[:, s, bass.ts(t, LF)]
                    # aw' = E*recipD + 1e-6 ; zsc = aw' * rq * exp(g)
                    aw1 = gjp.tile([128, LF], F32, tag="gj")
                    nc.scalar.activation(aw1[:], ef[:, t, soff:soff + LF],
                                         AF.Identity, scale=rd4[:, t:t + 1],
                                         bias=c1em6[:])
                    nc.vector.scalar_tensor_tensor(
                        slot, aw1[:], rq4[:, t:t + 1], gt[:, t, :],
                        op0=OP.mult, op1=OP.mult)
                # attn^T -> E^T = exp (bf16), selection-independent
                for u in range(4):
                    psT = eaT.tile([128, 512], F32, tag="aT")
                    lhsT2 = qk[qpp:qpp + 64, 4 + qpo,
                               bass.ds(b * N + u * 128, 128)]
                    rhsT2 = qk[qpp:qpp + 64, qpo, bass.ds(b * N, 512)]
                    nc.tensor.matmul(psT[:], lhsT2, rhsT2,
                                     start=True, stop=True)
                    nc.scalar.activation(e2t_all[:, s, u, :], psT[:], AF.Exp)
                # overlapped fixed-probe count sweeps for the selection
                if s < 8:
                    jc = gp.tile([128, 4 * LF], BF16, tag="jc")
                    nc.vector.tensor_scalar(jc[:], sc[:, s, :], thr0c[:], 0.0,
                                            op0=OP.is_ge, op1=OP.add,
                                            accum_out=cnt0[:, s:s + 1])
                    jc = gp.tile([128, 4 * LF], BF16, tag="jc")
                    nc.vector.tensor_scalar(jc[:], sc[:, s, :], thr1c[:], 0.0,
                                            op0=OP.is_ge, op1=OP.add,
                                            accum_out=cnt1[:, s:s + 1])
                else:
                    jc = gp.tile([128, 4 * LF], BF16, tag="jc")
                    nc.scalar.activation(jc[:], sc[:, s, :], AF.Sign,
                                         bias=nthr0c[:],
                                         accum_out=cnt0[:, s:s + 1])
                    jc = gp.tile([128, 4 * LF], BF16, tag="jc")
                    nc.scalar.activation(jc[:], sc[:, s, :], AF.Sign,
                                         bias=nthr1c[:],
                                         accum_out=cnt1[:, s:s + 1])

        # ---------- phase D: threshold selection (secant, 6 sweeps) ----------
        with tc.tile_pool(name="dps", bufs=2, space="PSUM") as dps, \
             tc.tile_pool(name="dbc", bufs=2, space="PSUM") as dbc, \
             tc.tile_pool(name="sel", bufs=1) as sel, \
             tc.tile_pool(name="junk", bufs=2) as jp, \
             tc.tile_pool(name="s16", bufs=8) as s16:
            cnt128 = sel.tile([128, NSL], F32)
            thrN = sel.tile([128, NSL], F32)
            t0 = sel.tile([1, NSL], F32)
            t1 = sel.tile([1, NSL], F32)
            tneg = sel.tile([1, NSL], F32)
            n0 = sel.tile([1, NSL], F32)
            n1 = sel.tile([1, NSL], F32)
            nc.vector.memset(t0[:], math.exp(-9.10))
            nc.vector.memset(t1[:], math.exp(-8.80))

            def count_into(ndst):
                for s in range(8):
                    jk = jp.tile([128, 4 * LF], BF16, tag="junk")
                    nc.vector.tensor_scalar(jk[:], sc[:, s, :],
                                            thr[:, s:s + 1], 0.0,
                                            op0=OP.is_ge, op1=OP.add,
                                            accum_out=cnt128[:, s:s + 1])
                for s in range(8, NSL):
                    jk2 = jp.tile([128, 4 * LF], BF16, tag="junk2")
                    nc.scalar.activation(jk2[:], sc[:, s, :], AF.Sign,
                                         bias=thrN[:, s:s + 1],
                                         accum_out=cnt128[:, s:s + 1])
                cp_ = dps.tile([1, NSL], F32, tag="cnt")
                nc.tensor.matmul(cp_[:], ones_col[:], cnt128[:],
                                 start=True, stop=True)
                nc.vector.tensor_tensor(ndst, cp_[:], a16[:], op=OP.mult)
                nc.vector.tensor_tensor(ndst, ndst, b16[:], op=OP.add)

            def bcast(tsrc):
                bp = dbc.tile([128, NSL], F32, tag="bc")
                nc.tensor.matmul(bp[:], ones_row[:], tsrc, start=True,
                                 stop=True)
                nc.vector.tensor_copy(thr[:], bp[:])
                nc.vector.tensor_scalar(tneg[:], tsrc, -1.0, None,
                                        op0=OP.mult)
                bp2 = dbc.tile([128, NSL], F32, tag="bc2")
                nc.tensor.matmul(bp2[:], ones_row[:], tneg[:], start=True,
                                 stop=True)
                nc.vector.tensor_copy(thrN[:], bp2[:])

            for csrc, ndst in ((cnt0, n0), (cnt1, n1)):
                cp_ = dps.tile([1, NSL], F32, tag="cnt")
                nc.tensor.matmul(cp_[:], ones_col[:], csrc[:],
                                 start=True, stop=True)
                nc.vector.tensor_tensor(ndst[:], cp_[:], a16[:], op=OP.mult)
                nc.vector.tensor_tensor(ndst[:], ndst[:], b16[:], op=OP.add)
            for r in range(3):
                d = s16.tile([1, NSL], F32, tag="d")
                nc.vector.tensor_tensor(d[:], n1[:], n0[:], op=OP.subtract)
                dsq = s16.tile([1, NSL], F32, tag="dsq")
                nc.vector.tensor_tensor(dsq[:], d[:], d[:], op=OP.mult)
                msk = s16.tile([1, NSL], F32, tag="msk")
                nc.vector.tensor_scalar(msk[:], dsq[:], 1.0, None,
                                        op0=OP.is_lt)
                nc.vector.tensor_tensor(d[:], d[:], msk[:], op=OP.subtract)
                rcd = s16.tile([1, NSL], F32, tag="rcd")
                nc.vector.reciprocal(rcd[:], d[:])
                dt = s16.tile([1, NSL], F32, tag="dt")
                nc.vector.tensor_tensor(dt[:], t1[:], t0[:], op=OP.subtract)
                nk = s16.tile([1, NSL], F32, tag="nk")
                nc.vector.tensor_scalar(nk[:], n1[:], KTARG, None,
                                        op0=OP.subtract)
                nc.vector.tensor_tensor(nk[:], nk[:], dt[:], op=OP.mult)
                nc.vector.tensor_tensor(nk[:], nk[:], rcd[:], op=OP.mult)
                t2 = s16.tile([1, NSL], F32, tag="t2")
                nc.vector.tensor_tensor(t2[:], t1[:], nk[:], op=OP.subtract)
                nc.vector.tensor_scalar(t2[:], t2[:], math.exp(-8.0),
                                        math.exp(-10.0),
                                        op0=OP.min, op1=OP.max)
                nc.vector.tensor_copy(t0[:], t1[:])
                nc.vector.tensor_copy(n0[:], n1[:])
                nc.vector.tensor_copy(t1[:], t2[:])
                bcast(t1[:])
                count_into(n1[:])
            if dbg is not None and "dbg_n" in dbg:
                nn = sel.tile([1, 2 * NSL], F32)
                nc.vector.tensor_copy(nn[:, :NSL], n0[:])
                nc.vector.tensor_copy(nn[:, NSL:], n1[:])
                nc.sync.dma_start(dbg["dbg_n"], nn[:])

        if dbg is not None:
            srcs = {"dbg_sc": sc, "dbg_thr": thr, "dbg_so": so_all,
                    "dbg_qk": qk}
            for name, t_ in srcs.items():
                if name in dbg:
                    nc.sync.dma_start(dbg[name], t_[:].bitcast(F32))

        # ---------- phase E: masked softmax + attn@v (transposed) ----------
        obtp = ctx.enter_context(tc.tile_pool(name="obt", bufs=1))
        obt = obtp.tile([128, 4, TOK], BF16)      # attn-out^T [c, tok]
        with tc.tile_pool(name="ekt", bufs=4, space="PSUM") as ektp, \
             tc.tile_pool(name="ed2", bufs=2, space="PSUM") as ed2p, \
             tc.tile_pool(name="epo", bufs=2, space="PSUM") as epop, \
             tc.tile_pool(name="ek01", bufs=2) as k01p, \
             tc.tile_pool(name="ekts", bufs=2) as ktsp, \
             tc.tile_pool(name="est", bufs=2) as est:
            for s in range(NSL):
                b, h = divmod(s, H)
                qpo, qpp = h // 2, 64 * (h % 2)
                e2t = e2t_all[:, s]
                # keep mask (natural), 0/1 bf16
                k01 = k01p.tile([128, 4, LF], BF16, tag="k01")
                for t in range(4):
                    nc.vector.tensor_scalar(k01[:, t, :],
                                            sc[:, s, bass.ts(t, LF)],
                                            thr[:, s:s + 1], None,
                                            op0=OP.is_lt)
                # transpose keep blocks (PE for half, DMA xbar for half),
                # multiply into the diag quadrants of E^T
                for t in range(4):
                    qoff = 0 if t < 2 else LF     # qrow quadrant offset
                    for uu in range(2):
                        u = 2 * (t // 2) + uu
                        dst = e2t[:, u, qoff + (t % 2) * 128:
                                  qoff + (t % 2) * 128 + 128]
                        ktp = ektp.tile([128, 128], BF16, tag="ktp")
                        nc.tensor.transpose(ktp[:],
                                            k01[:, t, bass.ts(uu, 128)],
                                            idbf[:])
                        nc.vector.tensor_tensor(dst, dst, ktp[:],
                                                op=OP.mult)
                # D2 row-sums via ones-matmul over kcol partitions
                d2 = ed2p.tile([1, N], F32, tag="d2")
                for u in range(4):
                    nc.tensor.matmul(d2[:], ones_col_bf[:], e2t[:, u, :],
                                     start=(u == 0), stop=(u == 3))
                d2sb = est.tile([1, N], F32, tag="d2sb")
                nc.scalar.copy(d2sb[:], d2[:])
                nc.gpsimd.dma_start(rd2_all[s:s + 1, :], d2sb[:])
                # out^T = v^T @ p^T  (accumulate over kcol chunks)
                po = epop.tile([64, N], F32, tag="po")
                for u in range(4):
                    nc.tensor.matmul(po[:],
                                     vbf[:, 4 * b + u, bass.ts(h, 64)],
                                     e2t[:, u, :],
                                     start=(u == 0), stop=(u == 3))
                nc.scalar.copy(obt[qpp:qpp + 64, qpo, bass.ds(b * N, 512)],
                               po[:])
            # batched reciprocal of all row-sums (16 partitions in parallel)
            nc.vector.reciprocal(rd2_all[:], rd2_all[:])

        # ---------- phase F: output projection ----------
        with tc.tile_pool(name="fps", bufs=4, space="PSUM") as fp, \
             tc.tile_pool(name="o2", bufs=1) as o2p:
            for ct in range(4):
                for bb in range(2):
                    rdbf = fp.tile([128, 512], F32, tag="rdbf")
                    nc.tensor.matmul(rdbf[:], selc[:, ct * 2 + bb, :],
                                     rd2_all[:], start=True, stop=True)
                    dst = obt[:, ct, bass.ts(bb, 512)]
                    nc.vector.tensor_tensor(dst, dst, rdbf[:], op=OP.mult)
            o2 = o2p.tile([128, 4, TOK], F32)     # proj out^T [oc, tok]
            for oc in range(4):
                for tch in range(2):
                    ps = fp.tile([128, 512], F32, tag="pj")
                    for ct in range(4):
                        nc.tensor.matmul(ps[:], wpb[:, ct, bass.ts(oc, 128)],
                                         obt[:, ct, bass.ts(tch, 512)],
                                         start=(ct == 0), stop=(ct == 3))
                    if (oc + tch) % 2:
                        nc.scalar.copy(o2[:, oc, bass.ts(tch, 512)], ps[:])
                    else:
                        nc.vector.tensor_copy(o2[:, oc, bass.ts(tch, 512)],
                                              ps[:])
            nc.sync.dma_start(out_d.rearrange("(k p) t -> p k t", p=128),
                              o2[:])


def _sel_mask():
    sel = np.zeros((16, 8, 128), np.float32)
    for ct in range(4):
        for bb in range(2):
            for p in range(128):
                sel[bb * 8 + 2 * ct + p // 64, ct * 2 + bb, p] = 1.0
    return sel


def _prep_inputs(x, w_qkv, w_proj):
    gum = _gumbel_full()
    eye = _eye_mask()
    sel = _sel_mask()
    wqt = np.ascontiguousarray(w_qkv.T).astype(np.float32).copy()
    wqt[:, :C] *= HD ** -0.5
    wpt = np.ascontiguousarray(w_proj.T).astype(np.float32)
    in_maps = []
    for i in range(NCORES):
        xs = np.ascontiguousarray(
            x[BPC * i:BPC * (i + 1)].reshape(TOK, C).T)
        gs = np.ascontiguousarray(gum[NSL * i:NSL * (i + 1)])
        in_maps.append({
            "xt": xs, "wqt": wqt, "wpt": wpt, "gum": gs, "eye": eye,
            "sel": sel,
        })
    return in_maps


def kernel(x, w_qkv, w_proj, b_proj, _trace=False, _tracedir=None):
    x = np.asarray(x, np.float32)
    w_qkv = np.asarray(w_qkv, np.float32)
    w_proj = np.asarray(w_proj, np.float32)
    b_proj = np.asarray(b_proj, np.float32)
    nc = _build()
    in_maps = _prep_inputs(x, w_qkv, w_proj)
    kw = {}
    if _trace:
        kw = dict(trace=True, tmpdir=_tracedir)
    res = run_bass_kernel_spmd(nc, in_maps, core_ids=list(range(NCORES)), **kw)
    out = np.empty((B, N, C), np.float32)
    for i in range(NCORES):
        ot = np.asarray(res.results[i]["outt"])     # [C, TOK]
        out[BPC * i:BPC * (i + 1)] = ot.T.reshape(BPC, N, C)
    out += b_proj
    if _trace:
        return out, res
    return out
